# revision 45
# baseline (speedup 1.0000x reference)
"""DeepSAT GNN message-passing kernel for 8 TRN2 NeuronCores.

Mathematical restructuring exploited (validated vs reference to ~1e-7):
  * Every node updates exactly once, at iteration l == forward_level[node],
    always starting from the same constant hidden state h_init (broadcast of
    emd_w[:,0]+emd_b).  Hence gh = w_hh @ h_init + b_hh is one constant vector.
  * Sources that have not updated yet (src_level == 0 or src_level >= tgt_level)
    contribute the constant agg(h_init) to the aggregation -> those edges fold
    into per-target integer counts (host-side index work).
  * The aggregation transform is linear, so msg @ w_ih_m.T fuses into
    s @ (w_ih_m @ agg_w).T  where s = sum of raw h[src] over active edges.
  * The per-target affine terms (counts, degree, bias) ride along as 3 extra
    rows of a K=19 matmul operand together with x.T.
  * The 3-layer MLP head is fused into the level loop (h tiles are consumed
    feature-major straight out of the GRU), so the last level needs no
    AllGather and no h table entry.

Distribution (graph parallel, SPMD — one program, per-core data via inputs):
  * Nodes renumbered by (level, owning core, position); each level's nodes
    split into 8 equal chunks, one per core.
  * Replicated h tables split into window tensors of <= 32767 rows (2 levels
    each) so rows are addressable by int16 dma_gather indices.
  * Edges sharded by target core, grouped by (target level, source window);
    one dma_gather per group; segment-sum via one-hot matmuls into PSUM.
  * After each level, updated h rows are AllGather'd into the level's window
    tensor slice.
  * Gathers (and their segment-sum matmuls) for windows finalized 2+ levels
    ago are issued on the Pool queue BEFORE the previous level's AllGather,
    so they overlap it; only the newest window's gather waits.
"""

import os

import numpy as np

import concourse.bacc as bacc
import concourse.bass as bass
import concourse.mybir as mybir
import concourse.tile as tile
from concourse import library_config
from concourse.bass_utils import run_bass_kernel_spmd
from concourse.tile_rust import add_dep_helper

NCORES = 8
TILE_T = 512   # targets per compute tile (one PSUM bank of fp32)
ECHUNK = 128   # edges per segsum matmul chunk (K partition dim)
WIN_ROWS = 32767  # int16 dma_gather index limit per window tensor

F32 = mybir.dt.float32
H_DT = mybir.dt.float16   # h storage + compute dtype (table, gather, gates)
H_NP = np.float16

PAD_TOFF = 65504.0  # one-hot scalar for inactive edges: never equals iota


def _host_prep(x, edge_index, forward_level, L):
    """Integer/index preprocessing: renumbering, windows, edge groups,
    per-core input arrays."""
    N, DX = x.shape
    lvl = np.asarray(forward_level).astype(np.int64)
    src = np.asarray(edge_index[0]).astype(np.int64)
    tgt = np.asarray(edge_index[1]).astype(np.int64)

    sl, tl = lvl[src], lvl[tgt]
    active = (sl >= 1) & (sl < tl) & (tl < L)
    const_e = (~active) & (tl >= 1) & (tl < L)

    deg = np.bincount(tgt, minlength=N).astype(np.float32)
    cnt = np.bincount(tgt[const_e], minlength=N).astype(np.float32)

    # nodes whose h is ever gathered (>=1 active out-edge); only they need
    # to be staged / AllGather'd / kept in window tables
    used = np.zeros(N, bool)
    used[np.unique(src[active])] = True

    core_of = np.zeros(N, np.int64)
    pos_of = np.zeros(N, np.int64)
    chunk = [0] * L
    stage_rows = [0] * L   # rows staged+AG'd per core per level (used prefix)
    nodes_by_level = []
    for l in range(L):
        nodes = np.flatnonzero(lvl == l)
        nodes_by_level.append(nodes)
        if l >= 1 and len(nodes):
            chunk[l] = (len(nodes) + NCORES - 1) // NCORES
            mx_used = 0
            for c in range(NCORES):
                seg = nodes[c * chunk[l]:(c + 1) * chunk[l]]
                u = used[seg]
                order = np.argsort(~u, kind="stable")
                core_of[seg] = c
                pos_of[seg[order]] = np.arange(len(seg))
                mx_used = max(mx_used, int(u.sum()))
            if 1 <= l <= L - 2:
                stage_rows[l] = mx_used

    # ---- window tensors over source levels 1..L-2 ----
    # few levels per window: every window except the newest is final before
    # the previous level's AllGather, so its gathers + segment-sums can be
    # issued ahead of the AG and overlap it; fewer levels per window =
    # more prefetchable edges but more (padded) gather groups
    WPL = int(os.environ.get("KWPL", "0"))
    win_of = [-1] * L        # level -> window index
    lvl_off = [0] * L        # level -> row offset inside its window
    win_rows = []            # window -> total rows
    win_nlvl = 0
    for l in range(1, L - 1):
        rl = NCORES * stage_rows[l]
        assert rl <= WIN_ROWS, "one level exceeds the int16 window"
        if (not win_rows or win_rows[-1] + rl > WIN_ROWS
                or (WPL and win_nlvl >= WPL)):
            win_rows.append(0)
            win_nlvl = 0
        win_of[l] = len(win_rows) - 1
        lvl_off[l] = win_rows[-1]
        win_rows[-1] += rl
        win_nlvl += 1
    NWIN = len(win_rows)

    # per-core staging rows for AG inputs (levels 1..L-2, local order)
    stg_start = [0] * L
    sr = 0
    for l in range(1, L - 1):
        stg_start[l] = sr
        sr += stage_rows[l]
    R_stg = max(sr, 1)

    # per-core owned output columns (levels 1..L-1)
    own_start = [0] * L
    oc = 0
    for l in range(1, L):
        own_start[l] = oc
        oc += chunk[l]
    OWN = max(oc, 1)

    ntiles = [0] * L
    for l in range(1, L):
        ntiles[l] = (chunk[l] + TILE_T - 1) // TILE_T if chunk[l] else 0

    # xaux [core, DX+3, OWN]: x.T rows, then cnt, deg, ones
    xaux = np.zeros((NCORES, DX + 3, OWN), H_NP)
    for l in range(1, L):
        nodes = nodes_by_level[l]
        if not len(nodes):
            continue
        c, p = core_of[nodes], pos_of[nodes]
        cols = own_start[l] + p
        xaux[c, :DX, cols] = np.asarray(x)[nodes].astype(H_NP)
        xaux[c, DX, cols] = cnt[nodes]
        xaux[c, DX + 1, cols] = deg[nodes]
        xaux[c, DX + 2, cols] = 1.0

    # ---- active edges -> (core, tgt level, src window) groups ----
    a_src, a_tgt = src[active], tgt[active]
    e_widx = (np.array(lvl_off)[lvl[a_src]]
              + core_of[a_src] * np.array(stage_rows)[lvl[a_src]]
              + pos_of[a_src]).astype(np.int64)
    # group axis g = 2*window + fresh: "fresh" edges (source exactly one
    # level below the target) are the ONLY ones that need the immediately
    # preceding AllGather; older-source groups in the same window tensor
    # can gather + segment-sum ahead of it
    e_w = np.array(win_of)[lvl[a_src]]
    e_fresh = (lvl[a_src] == lvl[a_tgt] - 1).astype(np.int64)
    e_g = e_w * 2 + e_fresh
    NG = 2 * NWIN
    e_core = core_of[a_tgt]
    e_lvl = lvl[a_tgt]
    e_tile = pos_of[a_tgt] // TILE_T
    e_toff = (pos_of[a_tgt] % TILE_T).astype(np.float32)

    order = np.lexsort((e_toff, e_tile, e_g, e_lvl, e_core))
    e_widx, e_g, e_core, e_lvl, e_tile, e_toff = (
        a[order] for a in (e_widx, e_g, e_core, e_lvl, e_tile, e_toff))

    # split points per (core, level, group)
    per = {}
    key = (e_core * L + e_lvl) * NG + e_g
    uk, ustart, ucnt = np.unique(key, return_index=True, return_counts=True)
    for k, s0, n in zip(uk, ustart, ucnt):
        g = int(k % NG)
        l = int((k // NG) % L)
        c = int(k // (NG * L))
        per[(c, l, g)] = (int(s0), int(n))

    # group schedule: (l, g) -> padded size, chunk count
    groups = {}   # (l, g) -> dict(num, nch, gcol, rows_avail)
    IDXCOLS = 0
    rows_written = [0] * NWIN   # rows present in window w before level l
    rows_hist = {}              # (l, w) -> rows readable at level l
    for l in range(1, L):
        for w in range(NWIN):
            rows_hist[(l, w)] = rows_written[w]
        if 1 <= l <= L - 2 and chunk[l]:
            rows_written[win_of[l]] += NCORES * stage_rows[l]
    for l in range(2, L):
        for g in range(NG):
            mx = max((per.get((c, l, g), (0, 0))[1] for c in range(NCORES)),
                     default=0)
            if mx == 0:
                continue
            nch = (mx + ECHUNK - 1) // ECHUNK
            w = g // 2
            # old groups (g even) only reference rows final before level l-1
            rows = rows_hist[(l, w)] if g % 2 else rows_hist[(l - 1, w)]
            groups[(l, g)] = dict(num=nch * ECHUNK, nch=nch, gcol=IDXCOLS,
                                  rows=rows)
            IDXCOLS += (nch * ECHUNK) // 16
    IDXCOLS = max(IDXCOLS, 1)

    # occurrences: (l, g, chunk k, tile t) present on any core
    occ_set = set()
    for (c, l, g), (s0, n) in per.items():
        tiles_of = e_tile[s0:s0 + n]
        for k in range(groups[(l, g)]["nch"]):
            a, b = k * ECHUNK, min((k + 1) * ECHUNK, n)
            if a >= n:
                break
            for t in np.unique(tiles_of[a:b]):
                occ_set.add((l, int(g), k, int(t)))
    occs = sorted(occ_set)
    occ_col = {o: i for i, o in enumerate(occs)}
    NOCC = max(len(occs), 1)
    occs_by_tile = {}
    for (l, g, k, t) in occs:
        occs_by_tile.setdefault((l, t), []).append((g, k, occ_col[(l, g, k, t)]))

    # per-core arrays
    gidx16 = np.zeros((NCORES, 128, IDXCOLS), np.int16)
    tofff = np.full((NCORES, 128, NOCC), PAD_TOFF, np.float32)
    for (c, l, gk), (s0, n) in per.items():
        g = groups[(l, gk)]
        num, nch, gcol = g["num"], g["nch"], g["gcol"]
        idxs = np.zeros(num, np.int16)
        idxs[:n] = e_widx[s0:s0 + n].astype(np.int16)
        wrapped = idxs.reshape(num // 16, 16).T  # [16, num/16]
        gidx16[c, :, gcol:gcol + num // 16] = np.tile(wrapped, (8, 1))
        tiles_of = e_tile[s0:s0 + n]
        toffs_of = e_toff[s0:s0 + n]
        for k in range(nch):
            a = k * ECHUNK
            b = min(a + ECHUNK, n)
            if a >= n:
                break
            for t in np.unique(tiles_of[a:b]):
                col = occ_col[(l, gk, k, t)]
                seg = np.full(ECHUNK, PAD_TOFF, np.float32)
                m = tiles_of[a:b] == t
                seg[:b - a][m] = toffs_of[a:b][m]
                tofff[c, :, col] = seg

    return dict(
        N=N, DX=DX, lvl=lvl, chunk=chunk, ntiles=ntiles,
        stage_rows=stage_rows,
        win_of=win_of, lvl_off=lvl_off, win_rows=win_rows, NWIN=NWIN, NG=NG,
        groups=groups, occs_by_tile=occs_by_tile,
        IDXCOLS=IDXCOLS, NOCC=NOCC, R_stg=R_stg,
        stg_start=stg_start, own_start=own_start, OWN=OWN,
        core_of=core_of, pos_of=pos_of,
        gidx16=gidx16, tofff=tofff, xaux=xaux,
    )


def _build_program(prep, consts, L, DH, DM):
    """Build the SPMD Bass program (identical across cores)."""
    DX = prep["DX"]
    OWN, IDXCOLS, NOCC = prep["OWN"], prep["IDXCOLS"], prep["NOCC"]
    ntiles, chunk = prep["ntiles"], prep["chunk"]
    stage_rows = prep["stage_rows"]
    groups, occs_by_tile = prep["groups"], prep["occs_by_tile"]
    win_of, lvl_off, win_rows = prep["win_of"], prep["lvl_off"], prep["win_rows"]
    stg_start, own_start = prep["stg_start"], prep["own_start"]
    R_stg = prep["R_stg"]
    NWIN = prep["NWIN"]
    NG = prep["NG"]
    OUT_COLS = 1 + OWN
    G3 = 3 * DH

    nc = bacc.Bacc("TRN2", target_bir_lowering=False, debug=False)

    gidx_t = nc.dram_tensor("gidx", [128, IDXCOLS], mybir.dt.int16,
                            kind="ExternalInput")
    toff_t = nc.dram_tensor("toff", [128, NOCC], F32, kind="ExternalInput")
    xaux_t = nc.dram_tensor("xaux", [DX + 3, OWN], H_DT, kind="ExternalInput")
    outT = nc.dram_tensor("outT", [1, OUT_COLS], F32, kind="ExternalOutput")

    wtbl = [nc.dram_tensor(f"wtbl{w}", [max(r, 1), DH], H_DT,
                           addr_space="Shared")
            for w, r in enumerate(win_rows)]
    h_stage = nc.dram_tensor("h_stage", [R_stg, DH], H_DT)

    WcT_c = nc.inline_tensor(consts["WcT"], "WcT")
    WxT_c = nc.inline_tensor(consts["WxT"], "WxT")
    ghnd_c = nc.inline_tensor(consts["ghnd"], "ghnd")
    hinit_c = nc.inline_tensor(consts["hinit"], "hinit")
    hinit32_c = nc.inline_tensor(consts["hinit32"], "hinit32")
    W1T_c = nc.inline_tensor(consts["W1T"], "W1T")
    W2T_c = nc.inline_tensor(consts["W2T"], "W2T")
    W3T_c = nc.inline_tensor(consts["W3T"], "W3T")
    b1_c = nc.inline_tensor(consts["b1"], "b1")
    b2_c = nc.inline_tensor(consts["b2"], "b2")
    iota_c = nc.inline_tensor(
        np.tile(np.arange(TILE_T, dtype=H_NP), (128, 1)), "iota")
    ident_c = nc.inline_tensor(np.eye(128, dtype=H_NP), "ident")
    b3f = float(consts["b3"])
    RG = [list(range(NCORES))]
    AF = mybir.ActivationFunctionType
    ALU = mybir.AluOpType

    with tile.TileContext(nc, num_cores=NCORES) as tc:
        with tc.tile_pool(name="cst", bufs=1) as cst, \
             tc.tile_pool(name="gat", bufs=int(os.environ.get("KGAT", "16"))) as gat, \
             tc.tile_pool(name="wrk", bufs=2) as wrk, \
             tc.tile_pool(name="acc", bufs=5) as acc, \
             tc.tile_pool(name="hp", bufs=5) as hp, \
             tc.tile_pool(name="psA", bufs=2, space="PSUM") as psA, \
             tc.tile_pool(name="ps", bufs=1, space="PSUM") as ps:

            nc.gpsimd.load_library(library_config.mlp)

            # ---- constants to SBUF ----
            def cload(name, src, shape, dtype=F32):
                t = cst.tile(shape, dtype, tag=name)
                nc.sync.dma_start(out=t[:], in_=src[:, :])
                return t

            iota = cload("iota", iota_c, [128, TILE_T], H_DT)
            ident = cload("ident", ident_c, [128, 128], H_DT)
            WcT = cload("WcT", WcT_c, [DH, G3], H_DT)
            WxT = cload("WxT", WxT_c, [DX + 3, G3], H_DT)
            ghnd = cload("ghnd", ghnd_c, [DH, DH], H_DT)
            hinit = cload("hinit", hinit_c, [DH, 1], H_DT)
            hinit32 = cload("hinit32", hinit32_c, [DH, 1])
            W1T = cload("W1T", W1T_c, [DH, DM], H_DT)
            W2T = cload("W2T", W2T_c, [DM, DM], H_DT)
            W3T = cload("W3T", W3T_c, [DM, 1], H_DT)
            b1 = cload("b1", b1_c, [DM, 1])
            b2 = cload("b2", b2_c, [DM, 1])
            gidx_sb = cload("gidx", gidx_t, [128, IDXCOLS], mybir.dt.int16)
            toff_sb = cload("toff", toff_t, [128, NOCC])
            xauxsb = cload("xauxsb", xaux_t, [DX + 3, OWN], H_DT)
            obuf = cst.tile([1, OUT_COLS], F32, tag="obuf")

            RELU_DVE = os.environ.get("KRELU", "act") == "dve"
            OUT_DVE = os.environ.get("KOUT", "dve") == "dve"

            def mlp(hT_sb, n_t, out_col):
                # z1 / z2 / out share one PSUM bank on disjoint partitions;
                # results accumulate in obuf (one outT DMA at the end).
                mp = ps.tile([2 * DM + 1, TILE_T], F32, tag="mlp")
                nc.tensor.matmul(mp[0:DM, :n_t], lhsT=W1T[:], rhs=hT_sb,
                                 start=True, stop=True)
                z1s = wrk.tile([DM, TILE_T], H_DT, tag="z1s")
                if RELU_DVE:
                    nc.vector.tensor_scalar(out=z1s[:, :n_t],
                                            in0=mp[0:DM, :n_t],
                                            scalar1=b1[:, 0:1], scalar2=0.0,
                                            op0=ALU.add, op1=ALU.max)
                else:
                    nc.scalar.activation(out=z1s[:, :n_t], in_=mp[0:DM, :n_t],
                                         func=AF.Relu, bias=b1[:, 0:1])
                nc.tensor.matmul(mp[DM:2 * DM, :n_t], lhsT=W2T[:],
                                 rhs=z1s[:, :n_t], start=True, stop=True)
                z2s = wrk.tile([DM, TILE_T], H_DT, tag="z2s")
                if RELU_DVE:
                    nc.vector.tensor_scalar(out=z2s[:, :n_t],
                                            in0=mp[DM:2 * DM, :n_t],
                                            scalar1=b2[:, 0:1], scalar2=0.0,
                                            op0=ALU.add, op1=ALU.max)
                else:
                    nc.scalar.activation(out=z2s[:, :n_t],
                                         in_=mp[DM:2 * DM, :n_t],
                                         func=AF.Relu, bias=b2[:, 0:1])
                nc.tensor.matmul(mp[2 * DM:2 * DM + 1, :n_t], lhsT=W3T[:],
                                 rhs=z2s[:, :n_t], start=True, stop=True)
                if OUT_DVE:
                    nc.vector.tensor_scalar(
                        out=obuf[0:1, out_col:out_col + n_t],
                        in0=mp[2 * DM:2 * DM + 1, :n_t],
                        scalar1=b3f, scalar2=None, op0=ALU.add)
                else:
                    nc.scalar.activation(out=obuf[0:1, out_col:out_col + n_t],
                                         in_=mp[2 * DM:2 * DM + 1, :n_t],
                                         func=AF.Copy, bias=b3f)

            # output column 0: MLP(h_init) for never-updated nodes
            mlp(hinit[:, 0:1], 1, 0)

            KREPS = int(os.environ.get("KREPS", "1"))
            STAGE_MERGE = os.environ.get("KSTAGE", "merge") == "merge"
            STAGE_PSUM = os.environ.get("KSTAGE", "merge") == "psum"
            MLP_DEFER = os.environ.get("KMLPDEF", "1") == "1"

            # Early-ready producers (gathers on old windows, one-hots) must
            # acquire pool slots roughly in program order or the scheduler's
            # slot waits can form cycles.  Anchor them to recent per-tile
            # instructions.
            anchors = []   # one per processed tile: the sigmoid activation
            gchain = []
            GCHAIN = 4
            gtiles = {}    # (l, w) -> (gather tile, gather ins)

            def emit_gather(l, g):
                gr = groups[(l, g)]
                num, nch, gcol = gr["num"], gr["nch"], gr["gcol"]
                gt = gat.tile([128, nch * DH], H_DT, tag="g")
                gi = nc.gpsimd.dma_gather(
                    gt[:].rearrange("p (q e) -> p q e", e=DH),
                    wtbl[g // 2][0:gr["rows"], :],
                    gidx_sb[:, gcol:gcol + num // 16],
                    num, num, DH)
                gchain.append(gi.ins)
                if len(gchain) > GCHAIN:
                    add_dep_helper(gchain[-1], gchain[-1 - GCHAIN],
                                   sync=True, reason="gather slot pacing")
                if anchors:
                    add_dep_helper(gi.ins, anchors[-1], sync=True,
                                   reason="gather level pacing")
                gtiles[(l, g)] = (gt, gi.ins)

            for rep in range(KREPS):
             for l in range(1, L):
                # gathers not issued during the previous level (fresh groups
                # need AG(l-1); everything at a rep's first levels)
                for g in range(NG):
                    if (l, g) in groups and (l, g) not in gtiles:
                        emit_gather(l, g)

                def occ_mms(occ_list, n_t, l):
                    sTp = psA.tile([DH, TILE_T], F32, tag="sT")
                    for i, (g, k, col) in enumerate(occ_list):
                        oh = wrk.tile([ECHUNK, TILE_T], H_DT, tag="oh")
                        ohi = nc.vector.tensor_scalar(
                            out=oh[:, :n_t], in0=iota[:, :n_t],
                            scalar1=toff_sb[:, col:col + 1], scalar2=None,
                            op0=ALU.is_equal)
                        add_dep_helper(ohi.ins, gtiles[(l, g)][1],
                                       sync=True, reason="onehot pacing")
                        nc.tensor.matmul(
                            sTp[:, :n_t],
                            lhsT=gtiles[(l, g)][0][:, k * DH:(k + 1) * DH],
                            rhs=oh[:, :n_t],
                            start=(i == 0), stop=(i == len(occ_list) - 1))
                    return sTp

                # PASS A: old-group segment-sums for ALL tiles first (only
                # fresh groups — source level == l-1 — depend on AG(l-1);
                # old groups overlap it.  Emitting any fresh one-hot earlier
                # would head-of-line-block the DVE queue.
                sAcc = {}
                for t in range(ntiles[l]):
                    n_t = min(TILE_T, chunk[l] - t * TILE_T)
                    old = [o for o in occs_by_tile.get((l, t), [])
                           if o[0] % 2 == 0]
                    if old:
                        sTp = occ_mms(old, n_t, l)
                        a = acc.tile([DH, TILE_T], H_DT, tag="sAcc")
                        nc.vector.tensor_copy(out=a[:, :n_t],
                                              in_=sTp[:, :n_t])
                        sAcc[t] = a

                # PASS B: newest-window segment-sums + GRU per tile
                pend_mlp = []
                for t in range(ntiles[l]):
                    n_t = min(TILE_T, chunk[l] - t * TILE_T)
                    new = [o for o in occs_by_tile.get((l, t), [])
                           if o[0] % 2 == 1]
                    sT_sb = None
                    if new:
                        sTp = occ_mms(new, n_t, l)
                        sT_sb = wrk.tile([DH, TILE_T], H_DT, tag="sTs")
                        if t in sAcc:
                            nc.vector.tensor_tensor(
                                out=sT_sb[:, :n_t], in0=sTp[:, :n_t],
                                in1=sAcc[t][:, :n_t], op=ALU.add)
                        else:
                            nc.vector.tensor_copy(out=sT_sb[:, :n_t],
                                                  in_=sTp[:, :n_t])
                    elif t in sAcc:
                        sT_sb = sAcc[t]

                    oc0 = own_start[l] + t * TILE_T
                    xa = xauxsb[:, oc0:oc0 + n_t]

                    # r gate at cols [0:n_t], z gate bank-aligned at
                    # [TILE_T : TILE_T+n_t] (a matmul must not cross banks)
                    girz = ps.tile([DH, 2 * TILE_T], F32, tag="girz")
                    for gi_g in range(2):
                        gsl = slice(gi_g * TILE_T, gi_g * TILE_T + n_t)
                        wsl = slice(gi_g * DH, (gi_g + 1) * DH)
                        if sT_sb is not None:
                            nc.tensor.matmul(girz[:, gsl], lhsT=WcT[:, wsl],
                                             rhs=sT_sb[:, :n_t],
                                             start=True, stop=False)
                        nc.tensor.matmul(girz[:, gsl], lhsT=WxT[:, wsl],
                                         rhs=xa,
                                         start=(sT_sb is None), stop=True)
                    gin = ps.tile([DH, TILE_T], F32, tag="gin")
                    if sT_sb is not None:
                        nc.tensor.matmul(gin[:, :n_t], lhsT=WcT[:, 2 * DH:G3],
                                         rhs=sT_sb[:, :n_t],
                                         start=True, stop=False)
                    nc.tensor.matmul(gin[:, :n_t], lhsT=WxT[:, 2 * DH:G3],
                                     rhs=xa,
                                     start=(sT_sb is None), stop=False)

                    rz = wrk.tile([DH, 2 * TILE_T], H_DT, tag="rz")
                    if n_t == TILE_T:
                        sgi = nc.scalar.activation(out=rz[:, :2 * TILE_T],
                                                   in_=girz[:, :2 * TILE_T],
                                                   func=AF.Sigmoid)
                    else:
                        nc.scalar.activation(
                            out=rz[:, TILE_T:TILE_T + n_t],
                            in_=girz[:, TILE_T:TILE_T + n_t], func=AF.Sigmoid)
                        sgi = nc.scalar.activation(out=rz[:, :n_t],
                                                   in_=girz[:, :n_t],
                                                   func=AF.Sigmoid)
                    anchors.append(sgi.ins)
                    nc.tensor.matmul(gin[:, :n_t], lhsT=ghnd[:],
                                     rhs=rz[:, :n_t], start=False, stop=True)
                    n_sb = wrk.tile([DH, TILE_T], H_DT, tag="n")
                    nc.scalar.activation(out=n_sb[:, :n_t], in_=gin[:, :n_t],
                                         func=AF.Tanh)
                    t3 = wrk.tile([DH, TILE_T], H_DT, tag="t3")
                    nc.vector.tensor_scalar(out=t3[:, :n_t], in0=n_sb[:, :n_t],
                                            scalar1=hinit32[:, 0:1], scalar2=None,
                                            op0=ALU.subtract)
                    t4 = wrk.tile([DH, TILE_T], H_DT, tag="t4")
                    nc.vector.tensor_tensor(out=t4[:, :n_t],
                                            in0=rz[:, TILE_T:TILE_T + n_t],
                                            in1=t3[:, :n_t],
                                            op=ALU.mult)
                    hT = hp.tile([DH, TILE_T], H_DT, tag="hT")
                    nc.vector.tensor_tensor(out=hT[:, :n_t], in0=n_sb[:, :n_t],
                                            in1=t4[:, :n_t],
                                            op=ALU.subtract)

                    # MLP head deferred past the AllGather: it has no
                    # downstream consumer until the final outT DMA, and
                    # emitting it here would delay the next tile's gate
                    # matmuls/activations on the in-order PE/Act queues
                    if MLP_DEFER:
                        pend_mlp.append((hT, n_t, 1 + oc0))
                    else:
                        mlp(hT[:, :n_t], n_t, 1 + oc0)

                    # stage only the used prefix (nodes some edge gathers)
                    stage_n = min(n_t, max(0, stage_rows[l] - t * TILE_T))
                    if l <= L - 2 and stage_n > 0:
                        trp = ps.tile([128, TILE_T], H_DT, tag="tr")
                        nch_t = (stage_n + 127) // 128
                        st = wrk.tile([128, TILE_T], H_DT, tag="st")
                        for ci in range(nch_t):
                            wdt = min(128, stage_n - ci * 128)
                            nc.tensor.transpose(
                                out=trp[:wdt, ci * 128:ci * 128 + 128],
                                in_=hT[:, ci * 128:ci * 128 + wdt],
                                identity=ident[:])
                        r0 = stg_start[l] + t * TILE_T
                        if STAGE_PSUM:
                            # DMA straight out of the PSUM transpose tile:
                            # removes the DVE copy from the pre-AG chain
                            if stage_n == TILE_T:
                                nc.sync.dma_start(
                                    out=h_stage[r0:r0 + TILE_T, :].rearrange(
                                        "(q p) e -> p q e", p=128),
                                    in_=trp[:].rearrange("p (q e) -> p q e",
                                                         e=DH))
                            else:
                                for ci in range(nch_t):
                                    wdt = min(128, stage_n - ci * 128)
                                    nc.sync.dma_start(
                                        out=h_stage[r0 + ci * 128:
                                                    r0 + ci * 128 + wdt, :],
                                        in_=trp[:wdt,
                                                ci * 128:ci * 128 + 128])
                        elif STAGE_MERGE and stage_n == TILE_T:
                            nc.vector.tensor_copy(out=st[:, :TILE_T],
                                                  in_=trp[:, :TILE_T])
                            nc.sync.dma_start(
                                out=h_stage[r0:r0 + TILE_T, :].rearrange(
                                    "(q p) e -> p q e", p=128),
                                in_=st[:].rearrange("p (q e) -> p q e", e=DH))
                        else:
                            for ci in range(nch_t):
                                wdt = min(128, stage_n - ci * 128)
                                nc.vector.tensor_copy(
                                    out=st[:wdt, ci * 128:ci * 128 + 128],
                                    in_=trp[:wdt, ci * 128:ci * 128 + 128])
                                nc.sync.dma_start(
                                    out=h_stage[r0 + ci * 128:
                                                r0 + ci * 128 + wdt, :],
                                    in_=st[:wdt, ci * 128:ci * 128 + 128])

                # pre-issue next level's old-group gathers (sources at
                # levels <= l-1, already final): they overlap the AG on the
                # Pool queue; fresh groups wait until level l+1
                if l + 1 < L:
                    for g in range(0, NG, 2):
                        if (l + 1, g) in groups:
                            emit_gather(l + 1, g)

                if l <= L - 2 and stage_rows[l] > 0:
                    w = win_of[l]
                    o0 = lvl_off[l]
                    sr_l = stage_rows[l]
                    if os.environ.get("KSKIP_AG"):
                        # timing-skeleton mode: local copy instead of AG
                        # (results are wrong across cores; sim feedback only)
                        nc.sync.dma_start(
                            out=wtbl[w][o0:o0 + sr_l, :],
                            in_=h_stage[stg_start[l]:stg_start[l] + sr_l, :])
                    else:
                        nc.gpsimd.collective_compute(
                            "AllGather", mybir.AluOpType.bypass,
                            replica_groups=RG,
                            ins=[h_stage[stg_start[l]:stg_start[l] + sr_l, :]],
                            outs=[wtbl[w][o0:o0 + NCORES * sr_l, :]],
                        )

                # deferred MLP heads: fill the PE/Act/DVE queues while the
                # AllGather runs on the Pool queue
                for hT_p, n_t_p, col_p in pend_mlp:
                    mlp(hT_p[:, :n_t_p], n_t_p, col_p)

                # this level's gather tiles are consumed; drop the refs
                for key in [k for k in gtiles if k[0] == l]:
                    del gtiles[key]

             # end of rep: final outT DMA emitted once, after the last rep
            nc.sync.dma_start(out=outT[0:1, :], in_=obuf[:, :])

    nc.compile()
    return nc


def _make_consts(DX, DH, emd_w, emd_b, agg_w, agg_b, w_ih, w_hh, b_ih, b_hh,
                 mlp_w1, mlp_b1, mlp_w2, mlp_b2, mlp_w3, mlp_b3):
    h_init = emd_w[:, 0] + emd_b
    gh = w_hh @ h_init + b_hh
    Wm, Wx = w_ih[:, :DH], w_ih[:, DH:]
    W_comb = Wm @ agg_w
    u1 = Wm @ (agg_w @ h_init)
    u2 = Wm @ agg_b
    cb = b_ih.copy()
    cb[:2 * DH] += gh[:2 * DH]
    WxauxT = np.zeros((DX + 3, 3 * DH), np.float32)
    WxauxT[:DX] = Wx.T
    WxauxT[DX] = u1
    WxauxT[DX + 1] = u2
    WxauxT[DX + 2] = cb
    return dict(
        WcT=np.ascontiguousarray(W_comb.T).astype(H_NP),
        WxT=np.ascontiguousarray(WxauxT).astype(H_NP),
        ghnd=np.diag(gh[2 * DH:]).astype(H_NP),
        hinit=h_init[:, None].astype(H_NP),
        hinit32=h_init[:, None].astype(np.float32),
        W1T=np.ascontiguousarray(mlp_w1.T).astype(H_NP),
        W2T=np.ascontiguousarray(mlp_w2.T).astype(H_NP),
        W3T=np.ascontiguousarray(mlp_w3.T).astype(H_NP),
        b1=mlp_b1[:, None].astype(np.float32),
        b2=mlp_b2[:, None].astype(np.float32),
        b3=mlp_b3.reshape(-1)[0],
    )


def prepare(x, edge_index, forward_level, num_layers_f,
            emd_w, emd_b, agg_w, agg_b, w_ih, w_hh, b_ih, b_hh,
            mlp_w1, mlp_b1, mlp_w2, mlp_b2, mlp_w3, mlp_b3):
    """Host prep + program build; returns (nc, in_maps, assemble)."""
    x = np.asarray(x, np.float32)
    L = int(np.asarray(num_layers_f))
    N, DX = x.shape
    DH = np.asarray(agg_w).shape[0]
    DM = np.asarray(mlp_w1).shape[0]
    consts = _make_consts(
        DX, DH,
        np.asarray(emd_w, np.float32), np.asarray(emd_b, np.float32),
        np.asarray(agg_w, np.float32), np.asarray(agg_b, np.float32),
        np.asarray(w_ih, np.float32), np.asarray(w_hh, np.float32),
        np.asarray(b_ih, np.float32), np.asarray(b_hh, np.float32),
        np.asarray(mlp_w1, np.float32), np.asarray(mlp_b1, np.float32),
        np.asarray(mlp_w2, np.float32), np.asarray(mlp_b2, np.float32),
        np.asarray(mlp_w3, np.float32), np.asarray(mlp_b3, np.float32))

    prep = _host_prep(x, np.asarray(edge_index), forward_level, L)
    nc = _build_program(prep, consts, L, DH, DM)
    in_maps = [
        {"gidx": np.ascontiguousarray(prep["gidx16"][c]),
         "toff": np.ascontiguousarray(prep["tofff"][c]),
         "xaux": np.ascontiguousarray(prep["xaux"][c])}
        for c in range(NCORES)
    ]

    def assemble(outs):
        """outs: [NCORES] arrays of shape [1, OUT_COLS] -> full [N, 1]."""
        lvl = prep["lvl"]
        own_start = prep["own_start"]
        core_of, pos_of = prep["core_of"], prep["pos_of"]
        o = np.stack([np.asarray(outs[c]).reshape(-1) for c in range(NCORES)])
        out_full = np.empty((N, 1), np.float32)
        out_full[:, 0] = o[0, 0]
        upd = (lvl >= 1) & (lvl < L)
        vi = np.flatnonzero(upd)
        cols = np.array(own_start)[lvl[vi]] + pos_of[vi] + 1
        out_full[vi, 0] = o[core_of[vi], cols]
        return out_full

    return nc, in_maps, assemble


def build_exec(nc, in_maps):
    """Compile the program once into a reusable jitted executable with
    device-resident inputs (run_bass_kernel_spmd re-traces and re-uploads on
    every call, which dominates wall time under the axon tunnel)."""
    import jax
    from jax.sharding import Mesh, PartitionSpec, NamedSharding
    try:
        from jax.experimental.shard_map import shard_map
    except ImportError:
        from jax import shard_map
    from concourse import bass2jax
    from concourse.bass2jax import _bass_exec_p, partition_id_tensor

    bass2jax.install_neuronx_cc_hook()
    n_cores = NCORES
    pname = nc.partition_id_tensor.name if nc.partition_id_tensor else None
    in_names, out_names, out_avals, zero_outs = [], [], [], []
    for alloc in nc.m.functions[0].allocations:
        if not isinstance(alloc, mybir.MemoryLocationSet):
            continue
        name = alloc.memorylocations[0].name
        if alloc.kind == "ExternalInput":
            if name != pname:
                in_names.append(name)
        elif alloc.kind == "ExternalOutput":
            shape = tuple(alloc.tensor_shape)
            dtype = mybir.dt.np(alloc.dtype)
            out_avals.append(jax.core.ShapedArray(shape, dtype))
            zero_outs.append(np.zeros(shape, dtype))
            out_names.append(name)
    n_params = len(in_names)
    n_outs = len(out_avals)
    in_names_full = in_names + out_names + ([pname] if pname else [])

    def _body(*args):
        operands = list(args)
        if pname is not None:
            operands.append(partition_id_tensor())
        outs = _bass_exec_p.bind(
            *operands, out_avals=tuple(out_avals),
            in_names=tuple(in_names_full), out_names=tuple(out_names),
            lowering_input_output_aliases=(), sim_require_finite=True,
            sim_require_nnan=True, nc=nc)
        return tuple(outs)

    devices = jax.devices()[:n_cores]
    mesh = Mesh(np.asarray(devices), ("core",))
    in_specs = (PartitionSpec("core"),) * (n_params + n_outs)
    out_specs = (PartitionSpec("core"),) * len(out_names)
    donate = tuple(range(n_params, n_params + n_outs))
    sharded = jax.jit(shard_map(_body, mesh=mesh, in_specs=in_specs,
                                out_specs=out_specs, check_rep=False),
                      donate_argnums=donate, keep_unused=True)
    per_core = [[np.asarray(m[name]) for name in in_names] for m in in_maps]
    concat_in = [np.concatenate([per_core[c][i] for c in range(n_cores)],
                                axis=0) for i in range(n_params)]
    compiled = sharded.lower(
        *concat_in,
        *[np.zeros((n_cores * z.shape[0], *z.shape[1:]), z.dtype)
          for z in zero_outs]).compile()
    sh = NamedSharding(mesh, PartitionSpec("core"))
    dev_in = [jax.device_put(a, sh) for a in concat_in]
    jax.block_until_ready(dev_in)
    return dict(compiled=compiled, dev_in=dev_in, zero_outs=zero_outs,
                out_names=out_names, out_avals=out_avals, sh=sh,
                n_cores=n_cores)


def fresh_zeros(B):
    """Donated output buffers (device-resident) for one execution."""
    import jax
    zs = [jax.device_put(
        np.zeros((B["n_cores"] * z.shape[0], *z.shape[1:]), z.dtype), B["sh"])
        for z in B["zero_outs"]]
    jax.block_until_ready(zs)
    return zs


def run_exec(B, zs=None):
    """One kernel execution; returns per-core output dicts."""
    import jax
    if zs is None:
        zs = fresh_zeros(B)
    outs = B["compiled"](*B["dev_in"], *zs)
    jax.block_until_ready(outs)
    return [
        {name: np.asarray(outs[i]).reshape(B["n_cores"],
                                           *B["out_avals"][i].shape)[c]
         for i, name in enumerate(B["out_names"])}
        for c in range(B["n_cores"])
    ]


_CACHE = {}


def kernel(**inputs):
    key = tuple(sorted((k, tuple(np.asarray(v).shape))
                       for k, v in inputs.items()))
    ent = _CACHE.get(key)
    digest = None
    try:
        import hashlib
        h = hashlib.sha1()
        for k in sorted(inputs):
            h.update(np.ascontiguousarray(np.asarray(inputs[k])).tobytes())
        digest = h.hexdigest()
    except Exception:
        pass
    if ent is None or ent["digest"] != digest or digest is None:
        nc, in_maps, assemble = prepare(**inputs)
        try:
            B = build_exec(nc, in_maps)
        except Exception:
            B = None
        ent = {"nc": nc, "in_maps": in_maps, "assemble": assemble, "B": B,
               "digest": digest}
        _CACHE[key] = ent
    globals()["LAST_RUN"] = {
        "nc": ent["nc"], "in_maps": ent["in_maps"], "exec_time_ns": None,
        "B": ent["B"],
    }
    if ent["B"] is not None:
        outs = run_exec(ent["B"])
        return ent["assemble"]([outs[c]["outT"] for c in range(NCORES)])
    res = run_bass_kernel_spmd(ent["nc"], ent["in_maps"],
                               core_ids=list(range(NCORES)))
    return ent["assemble"]([res.results[c]["outT"] for c in range(NCORES)])


# revision 47
# speedup vs baseline: 1.0111x; 1.0111x over previous
"""DeepSAT GNN message-passing kernel for 8 TRN2 NeuronCores.

Mathematical restructuring exploited (validated vs reference to ~1e-7):
  * Every node updates exactly once, at iteration l == forward_level[node],
    always starting from the same constant hidden state h_init (broadcast of
    emd_w[:,0]+emd_b).  Hence gh = w_hh @ h_init + b_hh is one constant vector.
  * Sources that have not updated yet (src_level == 0 or src_level >= tgt_level)
    contribute the constant agg(h_init) to the aggregation -> those edges fold
    into per-target integer counts (host-side index work).
  * The aggregation transform is linear, so msg @ w_ih_m.T fuses into
    s @ (w_ih_m @ agg_w).T  where s = sum of raw h[src] over active edges.
  * The per-target affine terms (counts, degree, bias) ride along as 3 extra
    rows of a K=19 matmul operand together with x.T.
  * The 3-layer MLP head is fused into the level loop (h tiles are consumed
    feature-major straight out of the GRU), so the last level needs no
    AllGather and no h table entry.

Distribution (graph parallel, SPMD — one program, per-core data via inputs):
  * Nodes renumbered by (level, owning core, position); each level's nodes
    split into 8 equal chunks, one per core.
  * Replicated h tables split into window tensors of <= 32767 rows (2 levels
    each) so rows are addressable by int16 dma_gather indices.
  * Edges sharded by target core, grouped by (target level, source window);
    one dma_gather per group; segment-sum via one-hot matmuls into PSUM.
  * After each level, updated h rows are AllGather'd into the level's window
    tensor slice.
  * Gathers (and their segment-sum matmuls) for windows finalized 2+ levels
    ago are issued on the Pool queue BEFORE the previous level's AllGather,
    so they overlap it; only the newest window's gather waits.
"""

import os

import numpy as np

import concourse.bacc as bacc
import concourse.bass as bass
import concourse.mybir as mybir
import concourse.tile as tile
from concourse import library_config
from concourse.bass_utils import run_bass_kernel_spmd
from concourse.tile_rust import add_dep_helper

NCORES = 8
TILE_T = 512   # targets per compute tile (one PSUM bank of fp32)
ECHUNK = 128   # edges per segsum matmul chunk (K partition dim)
WIN_ROWS = 32767  # int16 dma_gather index limit per window tensor

F32 = mybir.dt.float32
H_DT = mybir.dt.float16   # h storage + compute dtype (table, gather, gates)
H_NP = np.float16

PAD_TOFF = 65504.0  # one-hot scalar for inactive edges: never equals iota


def _host_prep(x, edge_index, forward_level, L):
    """Integer/index preprocessing: renumbering, windows, edge groups,
    per-core input arrays."""
    N, DX = x.shape
    lvl = np.asarray(forward_level).astype(np.int64)
    src = np.asarray(edge_index[0]).astype(np.int64)
    tgt = np.asarray(edge_index[1]).astype(np.int64)

    sl, tl = lvl[src], lvl[tgt]
    active = (sl >= 1) & (sl < tl) & (tl < L)
    const_e = (~active) & (tl >= 1) & (tl < L)

    deg = np.bincount(tgt, minlength=N).astype(np.float32)
    cnt = np.bincount(tgt[const_e], minlength=N).astype(np.float32)

    # nodes whose h is ever gathered (>=1 active out-edge); only they need
    # to be staged / AllGather'd / kept in window tables
    used = np.zeros(N, bool)
    used[np.unique(src[active])] = True

    core_of = np.zeros(N, np.int64)
    pos_of = np.zeros(N, np.int64)
    chunk = [0] * L
    stage_rows = [0] * L   # rows staged+AG'd per core per level (used prefix)
    nodes_by_level = []
    for l in range(L):
        nodes = np.flatnonzero(lvl == l)
        nodes_by_level.append(nodes)
        if l >= 1 and len(nodes):
            chunk[l] = (len(nodes) + NCORES - 1) // NCORES
            mx_used = 0
            for c in range(NCORES):
                seg = nodes[c * chunk[l]:(c + 1) * chunk[l]]
                u = used[seg]
                order = np.argsort(~u, kind="stable")
                core_of[seg] = c
                pos_of[seg[order]] = np.arange(len(seg))
                mx_used = max(mx_used, int(u.sum()))
            if 1 <= l <= L - 2:
                stage_rows[l] = mx_used

    # ---- window tensors over source levels 1..L-2 ----
    # few levels per window: every window except the newest is final before
    # the previous level's AllGather, so its gathers + segment-sums can be
    # issued ahead of the AG and overlap it; fewer levels per window =
    # more prefetchable edges but more (padded) gather groups
    WPL = int(os.environ.get("KWPL", "0"))
    win_of = [-1] * L        # level -> window index
    lvl_off = [0] * L        # level -> row offset inside its window
    win_rows = []            # window -> total rows
    win_nlvl = 0
    for l in range(1, L - 1):
        rl = NCORES * stage_rows[l]
        assert rl <= WIN_ROWS, "one level exceeds the int16 window"
        if (not win_rows or win_rows[-1] + rl > WIN_ROWS
                or (WPL and win_nlvl >= WPL)):
            win_rows.append(0)
            win_nlvl = 0
        win_of[l] = len(win_rows) - 1
        lvl_off[l] = win_rows[-1]
        win_rows[-1] += rl
        win_nlvl += 1
    NWIN = len(win_rows)

    # per-core staging rows for AG inputs (levels 1..L-2, local order)
    stg_start = [0] * L
    sr = 0
    for l in range(1, L - 1):
        stg_start[l] = sr
        sr += stage_rows[l]
    R_stg = max(sr, 1)

    # per-core owned output columns (levels 1..L-1)
    own_start = [0] * L
    oc = 0
    for l in range(1, L):
        own_start[l] = oc
        oc += chunk[l]
    OWN = max(oc, 1)

    ntiles = [0] * L
    for l in range(1, L):
        ntiles[l] = (chunk[l] + TILE_T - 1) // TILE_T if chunk[l] else 0

    # xaux [core, DX+3, OWN]: x.T rows, then cnt, deg, ones
    xaux = np.zeros((NCORES, DX + 3, OWN), H_NP)
    for l in range(1, L):
        nodes = nodes_by_level[l]
        if not len(nodes):
            continue
        c, p = core_of[nodes], pos_of[nodes]
        cols = own_start[l] + p
        xaux[c, :DX, cols] = np.asarray(x)[nodes].astype(H_NP)
        xaux[c, DX, cols] = cnt[nodes]
        xaux[c, DX + 1, cols] = deg[nodes]
        xaux[c, DX + 2, cols] = 1.0

    # ---- active edges -> (core, tgt level, src window) groups ----
    a_src, a_tgt = src[active], tgt[active]
    e_widx = (np.array(lvl_off)[lvl[a_src]]
              + core_of[a_src] * np.array(stage_rows)[lvl[a_src]]
              + pos_of[a_src]).astype(np.int64)
    # group axis g = 2*window + fresh: "fresh" edges (source exactly one
    # level below the target) are the ONLY ones that need the immediately
    # preceding AllGather; older-source groups in the same window tensor
    # can gather + segment-sum ahead of it
    e_w = np.array(win_of)[lvl[a_src]]
    e_fresh = (lvl[a_src] == lvl[a_tgt] - 1).astype(np.int64)
    e_g = e_w * 2 + e_fresh
    NG = 2 * NWIN
    e_core = core_of[a_tgt]
    e_lvl = lvl[a_tgt]
    e_tile = pos_of[a_tgt] // TILE_T
    e_toff = (pos_of[a_tgt] % TILE_T).astype(np.float32)

    order = np.lexsort((e_toff, e_tile, e_g, e_lvl, e_core))
    e_widx, e_g, e_core, e_lvl, e_tile, e_toff = (
        a[order] for a in (e_widx, e_g, e_core, e_lvl, e_tile, e_toff))

    # split points per (core, level, group)
    per = {}
    key = (e_core * L + e_lvl) * NG + e_g
    uk, ustart, ucnt = np.unique(key, return_index=True, return_counts=True)
    for k, s0, n in zip(uk, ustart, ucnt):
        g = int(k % NG)
        l = int((k // NG) % L)
        c = int(k // (NG * L))
        per[(c, l, g)] = (int(s0), int(n))

    # group schedule: (l, g) -> padded size, chunk count
    groups = {}   # (l, g) -> dict(num, nch, gcol, rows_avail)
    IDXCOLS = 0
    rows_written = [0] * NWIN   # rows present in window w before level l
    rows_hist = {}              # (l, w) -> rows readable at level l
    for l in range(1, L):
        for w in range(NWIN):
            rows_hist[(l, w)] = rows_written[w]
        if 1 <= l <= L - 2 and chunk[l]:
            rows_written[win_of[l]] += NCORES * stage_rows[l]
    for l in range(2, L):
        for g in range(NG):
            mx = max((per.get((c, l, g), (0, 0))[1] for c in range(NCORES)),
                     default=0)
            if mx == 0:
                continue
            nch = (mx + ECHUNK - 1) // ECHUNK
            w = g // 2
            # old groups (g even) only reference rows final before level l-1
            rows = rows_hist[(l, w)] if g % 2 else rows_hist[(l - 1, w)]
            groups[(l, g)] = dict(num=nch * ECHUNK, nch=nch, gcol=IDXCOLS,
                                  rows=rows)
            IDXCOLS += (nch * ECHUNK) // 16
    IDXCOLS = max(IDXCOLS, 1)

    # occurrences: (l, g, chunk k, tile t) present on any core
    occ_set = set()
    for (c, l, g), (s0, n) in per.items():
        tiles_of = e_tile[s0:s0 + n]
        for k in range(groups[(l, g)]["nch"]):
            a, b = k * ECHUNK, min((k + 1) * ECHUNK, n)
            if a >= n:
                break
            for t in np.unique(tiles_of[a:b]):
                occ_set.add((l, int(g), k, int(t)))
    occs = sorted(occ_set)
    occ_col = {o: i for i, o in enumerate(occs)}
    NOCC = max(len(occs), 1)
    occs_by_tile = {}
    for (l, g, k, t) in occs:
        occs_by_tile.setdefault((l, t), []).append((g, k, occ_col[(l, g, k, t)]))

    # per-core arrays
    gidx16 = np.zeros((NCORES, 128, IDXCOLS), np.int16)
    tofff = np.full((NCORES, 128, NOCC), PAD_TOFF, np.float32)
    for (c, l, gk), (s0, n) in per.items():
        g = groups[(l, gk)]
        num, nch, gcol = g["num"], g["nch"], g["gcol"]
        idxs = np.zeros(num, np.int16)
        idxs[:n] = e_widx[s0:s0 + n].astype(np.int16)
        wrapped = idxs.reshape(num // 16, 16).T  # [16, num/16]
        gidx16[c, :, gcol:gcol + num // 16] = np.tile(wrapped, (8, 1))
        tiles_of = e_tile[s0:s0 + n]
        toffs_of = e_toff[s0:s0 + n]
        for k in range(nch):
            a = k * ECHUNK
            b = min(a + ECHUNK, n)
            if a >= n:
                break
            for t in np.unique(tiles_of[a:b]):
                col = occ_col[(l, gk, k, t)]
                seg = np.full(ECHUNK, PAD_TOFF, np.float32)
                m = tiles_of[a:b] == t
                seg[:b - a][m] = toffs_of[a:b][m]
                tofff[c, :, col] = seg

    return dict(
        N=N, DX=DX, lvl=lvl, chunk=chunk, ntiles=ntiles,
        stage_rows=stage_rows,
        win_of=win_of, lvl_off=lvl_off, win_rows=win_rows, NWIN=NWIN, NG=NG,
        groups=groups, occs_by_tile=occs_by_tile,
        IDXCOLS=IDXCOLS, NOCC=NOCC, R_stg=R_stg,
        stg_start=stg_start, own_start=own_start, OWN=OWN,
        core_of=core_of, pos_of=pos_of,
        gidx16=gidx16, tofff=tofff, xaux=xaux,
    )


def _build_program(prep, consts, L, DH, DM):
    """Build the SPMD Bass program (identical across cores)."""
    DX = prep["DX"]
    OWN, IDXCOLS, NOCC = prep["OWN"], prep["IDXCOLS"], prep["NOCC"]
    ntiles, chunk = prep["ntiles"], prep["chunk"]
    stage_rows = prep["stage_rows"]
    groups, occs_by_tile = prep["groups"], prep["occs_by_tile"]
    win_of, lvl_off, win_rows = prep["win_of"], prep["lvl_off"], prep["win_rows"]
    stg_start, own_start = prep["stg_start"], prep["own_start"]
    R_stg = prep["R_stg"]
    NWIN = prep["NWIN"]
    NG = prep["NG"]
    OUT_COLS = 1 + OWN
    G3 = 3 * DH

    nc = bacc.Bacc("TRN2", target_bir_lowering=False, debug=False)

    gidx_t = nc.dram_tensor("gidx", [128, IDXCOLS], mybir.dt.int16,
                            kind="ExternalInput")
    toff_t = nc.dram_tensor("toff", [128, NOCC], F32, kind="ExternalInput")
    xaux_t = nc.dram_tensor("xaux", [DX + 3, OWN], H_DT, kind="ExternalInput")
    outT = nc.dram_tensor("outT", [1, OUT_COLS], F32, kind="ExternalOutput")

    wtbl = [nc.dram_tensor(f"wtbl{w}", [max(r, 1), DH], H_DT,
                           addr_space="Shared")
            for w, r in enumerate(win_rows)]
    h_stage = nc.dram_tensor("h_stage", [R_stg, DH], H_DT)

    WcT_c = nc.inline_tensor(consts["WcT"], "WcT")
    WxT_c = nc.inline_tensor(consts["WxT"], "WxT")
    ghnd_c = nc.inline_tensor(consts["ghnd"], "ghnd")
    hinit_c = nc.inline_tensor(consts["hinit"], "hinit")
    hinit32_c = nc.inline_tensor(consts["hinit32"], "hinit32")
    W1T_c = nc.inline_tensor(consts["W1T"], "W1T")
    W2T_c = nc.inline_tensor(consts["W2T"], "W2T")
    W3T_c = nc.inline_tensor(consts["W3T"], "W3T")
    b1_c = nc.inline_tensor(consts["b1"], "b1")
    b2_c = nc.inline_tensor(consts["b2"], "b2")
    iota_c = nc.inline_tensor(
        np.tile(np.arange(TILE_T, dtype=H_NP), (128, 1)), "iota")
    ident_c = nc.inline_tensor(np.eye(128, dtype=H_NP), "ident")
    b3f = float(consts["b3"])
    RG = [list(range(NCORES))]
    AF = mybir.ActivationFunctionType
    ALU = mybir.AluOpType

    with tile.TileContext(nc, num_cores=NCORES) as tc:
        with tc.tile_pool(name="cst", bufs=1) as cst, \
             tc.tile_pool(name="gat", bufs=int(os.environ.get("KGAT", "16"))) as gat, \
             tc.tile_pool(name="wrk", bufs=2) as wrk, \
             tc.tile_pool(name="acc", bufs=5) as acc, \
             tc.tile_pool(name="hp", bufs=5) as hp, \
             tc.tile_pool(name="psA", bufs=2, space="PSUM") as psA, \
             tc.tile_pool(name="ps", bufs=1, space="PSUM") as ps:

            nc.gpsimd.load_library(library_config.mlp)

            # ---- constants to SBUF ----
            def cload(name, src, shape, dtype=F32):
                t = cst.tile(shape, dtype, tag=name)
                nc.sync.dma_start(out=t[:], in_=src[:, :])
                return t

            iota = cload("iota", iota_c, [128, TILE_T], H_DT)
            ident = cload("ident", ident_c, [128, 128], H_DT)
            WcT = cload("WcT", WcT_c, [DH, G3], H_DT)
            WxT = cload("WxT", WxT_c, [DX + 3, G3], H_DT)
            ghnd = cload("ghnd", ghnd_c, [DH, DH], H_DT)
            hinit = cload("hinit", hinit_c, [DH, 1], H_DT)
            hinit32 = cload("hinit32", hinit32_c, [DH, 1])
            W1T = cload("W1T", W1T_c, [DH, DM], H_DT)
            W2T = cload("W2T", W2T_c, [DM, DM], H_DT)
            W3T = cload("W3T", W3T_c, [DM, 1], H_DT)
            b1 = cload("b1", b1_c, [DM, 1])
            b2 = cload("b2", b2_c, [DM, 1])
            gidx_sb = cload("gidx", gidx_t, [128, IDXCOLS], mybir.dt.int16)
            toff_sb = cload("toff", toff_t, [128, NOCC])
            xauxsb = cload("xauxsb", xaux_t, [DX + 3, OWN], H_DT)
            obuf = cst.tile([1, OUT_COLS], F32, tag="obuf")

            RELU_DVE = os.environ.get("KRELU", "act") == "dve"
            OUT_DVE = os.environ.get("KOUT", "act") == "dve"

            def mlp(hT_sb, n_t, out_col):
                # z1 / z2 / out share one PSUM bank on disjoint partitions;
                # results accumulate in obuf (one outT DMA at the end).
                mp = ps.tile([2 * DM + 1, TILE_T], F32, tag="mlp")
                nc.tensor.matmul(mp[0:DM, :n_t], lhsT=W1T[:], rhs=hT_sb,
                                 start=True, stop=True)
                z1s = wrk.tile([DM, TILE_T], H_DT, tag="z1s")
                if RELU_DVE:
                    nc.vector.tensor_scalar(out=z1s[:, :n_t],
                                            in0=mp[0:DM, :n_t],
                                            scalar1=b1[:, 0:1], scalar2=0.0,
                                            op0=ALU.add, op1=ALU.max)
                else:
                    nc.scalar.activation(out=z1s[:, :n_t], in_=mp[0:DM, :n_t],
                                         func=AF.Relu, bias=b1[:, 0:1])
                nc.tensor.matmul(mp[DM:2 * DM, :n_t], lhsT=W2T[:],
                                 rhs=z1s[:, :n_t], start=True, stop=True)
                z2s = wrk.tile([DM, TILE_T], H_DT, tag="z2s")
                if RELU_DVE:
                    nc.vector.tensor_scalar(out=z2s[:, :n_t],
                                            in0=mp[DM:2 * DM, :n_t],
                                            scalar1=b2[:, 0:1], scalar2=0.0,
                                            op0=ALU.add, op1=ALU.max)
                else:
                    nc.scalar.activation(out=z2s[:, :n_t],
                                         in_=mp[DM:2 * DM, :n_t],
                                         func=AF.Relu, bias=b2[:, 0:1])
                nc.tensor.matmul(mp[2 * DM:2 * DM + 1, :n_t], lhsT=W3T[:],
                                 rhs=z2s[:, :n_t], start=True, stop=True)
                if OUT_DVE:
                    nc.vector.tensor_scalar(
                        out=obuf[0:1, out_col:out_col + n_t],
                        in0=mp[2 * DM:2 * DM + 1, :n_t],
                        scalar1=b3f, scalar2=None, op0=ALU.add)
                else:
                    nc.scalar.activation(out=obuf[0:1, out_col:out_col + n_t],
                                         in_=mp[2 * DM:2 * DM + 1, :n_t],
                                         func=AF.Copy, bias=b3f)

            # output column 0: MLP(h_init) for never-updated nodes
            mlp(hinit[:, 0:1], 1, 0)

            KREPS = int(os.environ.get("KREPS", "1"))
            STAGE_MERGE = os.environ.get("KSTAGE", "merge") == "merge"
            STAGE_PSUM = os.environ.get("KSTAGE", "merge") == "psum"
            MLP_DEFER = os.environ.get("KMLPDEF", "1") == "1"

            # Early-ready producers (gathers on old windows, one-hots) must
            # acquire pool slots roughly in program order or the scheduler's
            # slot waits can form cycles.  Anchor them to recent per-tile
            # instructions.
            anchors = []   # one per processed tile: the sigmoid activation
            gchain = []
            GCHAIN = 4
            gtiles = {}    # (l, w) -> (gather tile, gather ins)

            def emit_gather(l, g):
                gr = groups[(l, g)]
                num, nch, gcol = gr["num"], gr["nch"], gr["gcol"]
                gt = gat.tile([128, nch * DH], H_DT, tag="g")
                gi = nc.gpsimd.dma_gather(
                    gt[:].rearrange("p (q e) -> p q e", e=DH),
                    wtbl[g // 2][0:gr["rows"], :],
                    gidx_sb[:, gcol:gcol + num // 16],
                    num, num, DH)
                gchain.append(gi.ins)
                if len(gchain) > GCHAIN:
                    add_dep_helper(gchain[-1], gchain[-1 - GCHAIN],
                                   sync=True, reason="gather slot pacing")
                if anchors:
                    add_dep_helper(gi.ins, anchors[-1], sync=True,
                                   reason="gather level pacing")
                gtiles[(l, g)] = (gt, gi.ins)

            for rep in range(KREPS):
             for l in range(1, L):
                # gathers not issued during the previous level (fresh groups
                # need AG(l-1); everything at a rep's first levels)
                for g in range(NG):
                    if (l, g) in groups and (l, g) not in gtiles:
                        emit_gather(l, g)

                def occ_mms(occ_list, n_t, l):
                    sTp = psA.tile([DH, TILE_T], F32, tag="sT")
                    for i, (g, k, col) in enumerate(occ_list):
                        oh = wrk.tile([ECHUNK, TILE_T], H_DT, tag="oh")
                        ohi = nc.vector.tensor_scalar(
                            out=oh[:, :n_t], in0=iota[:, :n_t],
                            scalar1=toff_sb[:, col:col + 1], scalar2=None,
                            op0=ALU.is_equal)
                        add_dep_helper(ohi.ins, gtiles[(l, g)][1],
                                       sync=True, reason="onehot pacing")
                        nc.tensor.matmul(
                            sTp[:, :n_t],
                            lhsT=gtiles[(l, g)][0][:, k * DH:(k + 1) * DH],
                            rhs=oh[:, :n_t],
                            start=(i == 0), stop=(i == len(occ_list) - 1))
                    return sTp

                # PASS A: old-group segment-sums for ALL tiles first (only
                # fresh groups — source level == l-1 — depend on AG(l-1);
                # old groups overlap it.  Emitting any fresh one-hot earlier
                # would head-of-line-block the DVE queue.
                sAcc = {}
                for t in range(ntiles[l]):
                    n_t = min(TILE_T, chunk[l] - t * TILE_T)
                    old = [o for o in occs_by_tile.get((l, t), [])
                           if o[0] % 2 == 0]
                    if old:
                        sTp = occ_mms(old, n_t, l)
                        a = acc.tile([DH, TILE_T], H_DT, tag="sAcc")
                        nc.vector.tensor_copy(out=a[:, :n_t],
                                              in_=sTp[:, :n_t])
                        sAcc[t] = a

                # PASS B: newest-window segment-sums + GRU per tile
                pend_mlp = []
                for t in range(ntiles[l]):
                    n_t = min(TILE_T, chunk[l] - t * TILE_T)
                    new = [o for o in occs_by_tile.get((l, t), [])
                           if o[0] % 2 == 1]
                    sT_sb = None
                    if new:
                        sTp = occ_mms(new, n_t, l)
                        sT_sb = wrk.tile([DH, TILE_T], H_DT, tag="sTs")
                        if t in sAcc:
                            nc.vector.tensor_tensor(
                                out=sT_sb[:, :n_t], in0=sTp[:, :n_t],
                                in1=sAcc[t][:, :n_t], op=ALU.add)
                        else:
                            nc.vector.tensor_copy(out=sT_sb[:, :n_t],
                                                  in_=sTp[:, :n_t])
                    elif t in sAcc:
                        sT_sb = sAcc[t]

                    oc0 = own_start[l] + t * TILE_T
                    xa = xauxsb[:, oc0:oc0 + n_t]

                    # r gate at cols [0:n_t], z gate bank-aligned at
                    # [TILE_T : TILE_T+n_t] (a matmul must not cross banks)
                    # dep-free WxT@xa accumulates FIRST so the in-order PE
                    # starts each tile's gates while DVE still merges sT
                    girz = ps.tile([DH, 2 * TILE_T], F32, tag="girz")
                    for gi_g in range(2):
                        gsl = slice(gi_g * TILE_T, gi_g * TILE_T + n_t)
                        wsl = slice(gi_g * DH, (gi_g + 1) * DH)
                        nc.tensor.matmul(girz[:, gsl], lhsT=WxT[:, wsl],
                                         rhs=xa,
                                         start=True, stop=(sT_sb is None))
                        if sT_sb is not None:
                            nc.tensor.matmul(girz[:, gsl], lhsT=WcT[:, wsl],
                                             rhs=sT_sb[:, :n_t],
                                             start=False, stop=True)
                    gin = ps.tile([DH, TILE_T], F32, tag="gin")
                    nc.tensor.matmul(gin[:, :n_t], lhsT=WxT[:, 2 * DH:G3],
                                     rhs=xa,
                                     start=True, stop=False)
                    if sT_sb is not None:
                        nc.tensor.matmul(gin[:, :n_t], lhsT=WcT[:, 2 * DH:G3],
                                         rhs=sT_sb[:, :n_t],
                                         start=False, stop=False)

                    rz = wrk.tile([DH, 2 * TILE_T], H_DT, tag="rz")
                    if n_t == TILE_T:
                        sgi = nc.scalar.activation(out=rz[:, :2 * TILE_T],
                                                   in_=girz[:, :2 * TILE_T],
                                                   func=AF.Sigmoid)
                    else:
                        nc.scalar.activation(
                            out=rz[:, TILE_T:TILE_T + n_t],
                            in_=girz[:, TILE_T:TILE_T + n_t], func=AF.Sigmoid)
                        sgi = nc.scalar.activation(out=rz[:, :n_t],
                                                   in_=girz[:, :n_t],
                                                   func=AF.Sigmoid)
                    anchors.append(sgi.ins)
                    nc.tensor.matmul(gin[:, :n_t], lhsT=ghnd[:],
                                     rhs=rz[:, :n_t], start=False, stop=True)
                    n_sb = wrk.tile([DH, TILE_T], H_DT, tag="n")
                    nc.scalar.activation(out=n_sb[:, :n_t], in_=gin[:, :n_t],
                                         func=AF.Tanh)
                    t3 = wrk.tile([DH, TILE_T], H_DT, tag="t3")
                    nc.vector.tensor_scalar(out=t3[:, :n_t], in0=n_sb[:, :n_t],
                                            scalar1=hinit32[:, 0:1], scalar2=None,
                                            op0=ALU.subtract)
                    t4 = wrk.tile([DH, TILE_T], H_DT, tag="t4")
                    nc.vector.tensor_tensor(out=t4[:, :n_t],
                                            in0=rz[:, TILE_T:TILE_T + n_t],
                                            in1=t3[:, :n_t],
                                            op=ALU.mult)
                    hT = hp.tile([DH, TILE_T], H_DT, tag="hT")
                    nc.vector.tensor_tensor(out=hT[:, :n_t], in0=n_sb[:, :n_t],
                                            in1=t4[:, :n_t],
                                            op=ALU.subtract)

                    # MLP head deferred past the AllGather: it has no
                    # downstream consumer until the final outT DMA, and
                    # emitting it here would delay the next tile's gate
                    # matmuls/activations on the in-order PE/Act queues
                    if MLP_DEFER:
                        pend_mlp.append((hT, n_t, 1 + oc0))
                    else:
                        mlp(hT[:, :n_t], n_t, 1 + oc0)

                    # stage only the used prefix (nodes some edge gathers)
                    stage_n = min(n_t, max(0, stage_rows[l] - t * TILE_T))
                    if l <= L - 2 and stage_n > 0:
                        trp = ps.tile([128, TILE_T], H_DT, tag="tr")
                        nch_t = (stage_n + 127) // 128
                        st = wrk.tile([128, TILE_T], H_DT, tag="st")
                        for ci in range(nch_t):
                            wdt = min(128, stage_n - ci * 128)
                            nc.tensor.transpose(
                                out=trp[:wdt, ci * 128:ci * 128 + 128],
                                in_=hT[:, ci * 128:ci * 128 + wdt],
                                identity=ident[:])
                        r0 = stg_start[l] + t * TILE_T
                        if STAGE_PSUM:
                            # DMA straight out of the PSUM transpose tile:
                            # removes the DVE copy from the pre-AG chain
                            if stage_n == TILE_T:
                                nc.sync.dma_start(
                                    out=h_stage[r0:r0 + TILE_T, :].rearrange(
                                        "(q p) e -> p q e", p=128),
                                    in_=trp[:].rearrange("p (q e) -> p q e",
                                                         e=DH))
                            else:
                                for ci in range(nch_t):
                                    wdt = min(128, stage_n - ci * 128)
                                    nc.sync.dma_start(
                                        out=h_stage[r0 + ci * 128:
                                                    r0 + ci * 128 + wdt, :],
                                        in_=trp[:wdt,
                                                ci * 128:ci * 128 + 128])
                        elif STAGE_MERGE and stage_n == TILE_T:
                            nc.vector.tensor_copy(out=st[:, :TILE_T],
                                                  in_=trp[:, :TILE_T])
                            nc.sync.dma_start(
                                out=h_stage[r0:r0 + TILE_T, :].rearrange(
                                    "(q p) e -> p q e", p=128),
                                in_=st[:].rearrange("p (q e) -> p q e", e=DH))
                        else:
                            for ci in range(nch_t):
                                wdt = min(128, stage_n - ci * 128)
                                nc.vector.tensor_copy(
                                    out=st[:wdt, ci * 128:ci * 128 + 128],
                                    in_=trp[:wdt, ci * 128:ci * 128 + 128])
                                nc.sync.dma_start(
                                    out=h_stage[r0 + ci * 128:
                                                r0 + ci * 128 + wdt, :],
                                    in_=st[:wdt, ci * 128:ci * 128 + 128])

                # pre-issue next level's old-group gathers (sources at
                # levels <= l-1, already final): they overlap the AG on the
                # Pool queue; fresh groups wait until level l+1
                if l + 1 < L:
                    for g in range(0, NG, 2):
                        if (l + 1, g) in groups:
                            emit_gather(l + 1, g)

                if l <= L - 2 and stage_rows[l] > 0:
                    w = win_of[l]
                    o0 = lvl_off[l]
                    sr_l = stage_rows[l]
                    if os.environ.get("KSKIP_AG"):
                        # timing-skeleton mode: local copy instead of AG
                        # (results are wrong across cores; sim feedback only)
                        nc.sync.dma_start(
                            out=wtbl[w][o0:o0 + sr_l, :],
                            in_=h_stage[stg_start[l]:stg_start[l] + sr_l, :])
                    else:
                        nc.gpsimd.collective_compute(
                            "AllGather", mybir.AluOpType.bypass,
                            replica_groups=RG,
                            ins=[h_stage[stg_start[l]:stg_start[l] + sr_l, :]],
                            outs=[wtbl[w][o0:o0 + NCORES * sr_l, :]],
                        )

                # deferred MLP heads: fill the PE/Act/DVE queues while the
                # AllGather runs on the Pool queue
                for hT_p, n_t_p, col_p in pend_mlp:
                    mlp(hT_p[:, :n_t_p], n_t_p, col_p)

                # this level's gather tiles are consumed; drop the refs
                for key in [k for k in gtiles if k[0] == l]:
                    del gtiles[key]

             # end of rep: final outT DMA emitted once, after the last rep
            nc.sync.dma_start(out=outT[0:1, :], in_=obuf[:, :])

    nc.compile()
    return nc


def _make_consts(DX, DH, emd_w, emd_b, agg_w, agg_b, w_ih, w_hh, b_ih, b_hh,
                 mlp_w1, mlp_b1, mlp_w2, mlp_b2, mlp_w3, mlp_b3):
    h_init = emd_w[:, 0] + emd_b
    gh = w_hh @ h_init + b_hh
    Wm, Wx = w_ih[:, :DH], w_ih[:, DH:]
    W_comb = Wm @ agg_w
    u1 = Wm @ (agg_w @ h_init)
    u2 = Wm @ agg_b
    cb = b_ih.copy()
    cb[:2 * DH] += gh[:2 * DH]
    WxauxT = np.zeros((DX + 3, 3 * DH), np.float32)
    WxauxT[:DX] = Wx.T
    WxauxT[DX] = u1
    WxauxT[DX + 1] = u2
    WxauxT[DX + 2] = cb
    return dict(
        WcT=np.ascontiguousarray(W_comb.T).astype(H_NP),
        WxT=np.ascontiguousarray(WxauxT).astype(H_NP),
        ghnd=np.diag(gh[2 * DH:]).astype(H_NP),
        hinit=h_init[:, None].astype(H_NP),
        hinit32=h_init[:, None].astype(np.float32),
        W1T=np.ascontiguousarray(mlp_w1.T).astype(H_NP),
        W2T=np.ascontiguousarray(mlp_w2.T).astype(H_NP),
        W3T=np.ascontiguousarray(mlp_w3.T).astype(H_NP),
        b1=mlp_b1[:, None].astype(np.float32),
        b2=mlp_b2[:, None].astype(np.float32),
        b3=mlp_b3.reshape(-1)[0],
    )


def prepare(x, edge_index, forward_level, num_layers_f,
            emd_w, emd_b, agg_w, agg_b, w_ih, w_hh, b_ih, b_hh,
            mlp_w1, mlp_b1, mlp_w2, mlp_b2, mlp_w3, mlp_b3):
    """Host prep + program build; returns (nc, in_maps, assemble)."""
    x = np.asarray(x, np.float32)
    L = int(np.asarray(num_layers_f))
    N, DX = x.shape
    DH = np.asarray(agg_w).shape[0]
    DM = np.asarray(mlp_w1).shape[0]
    consts = _make_consts(
        DX, DH,
        np.asarray(emd_w, np.float32), np.asarray(emd_b, np.float32),
        np.asarray(agg_w, np.float32), np.asarray(agg_b, np.float32),
        np.asarray(w_ih, np.float32), np.asarray(w_hh, np.float32),
        np.asarray(b_ih, np.float32), np.asarray(b_hh, np.float32),
        np.asarray(mlp_w1, np.float32), np.asarray(mlp_b1, np.float32),
        np.asarray(mlp_w2, np.float32), np.asarray(mlp_b2, np.float32),
        np.asarray(mlp_w3, np.float32), np.asarray(mlp_b3, np.float32))

    prep = _host_prep(x, np.asarray(edge_index), forward_level, L)
    nc = _build_program(prep, consts, L, DH, DM)
    in_maps = [
        {"gidx": np.ascontiguousarray(prep["gidx16"][c]),
         "toff": np.ascontiguousarray(prep["tofff"][c]),
         "xaux": np.ascontiguousarray(prep["xaux"][c])}
        for c in range(NCORES)
    ]

    def assemble(outs):
        """outs: [NCORES] arrays of shape [1, OUT_COLS] -> full [N, 1]."""
        lvl = prep["lvl"]
        own_start = prep["own_start"]
        core_of, pos_of = prep["core_of"], prep["pos_of"]
        o = np.stack([np.asarray(outs[c]).reshape(-1) for c in range(NCORES)])
        out_full = np.empty((N, 1), np.float32)
        out_full[:, 0] = o[0, 0]
        upd = (lvl >= 1) & (lvl < L)
        vi = np.flatnonzero(upd)
        cols = np.array(own_start)[lvl[vi]] + pos_of[vi] + 1
        out_full[vi, 0] = o[core_of[vi], cols]
        return out_full

    return nc, in_maps, assemble


def build_exec(nc, in_maps):
    """Compile the program once into a reusable jitted executable with
    device-resident inputs (run_bass_kernel_spmd re-traces and re-uploads on
    every call, which dominates wall time under the axon tunnel)."""
    import jax
    from jax.sharding import Mesh, PartitionSpec, NamedSharding
    try:
        from jax.experimental.shard_map import shard_map
    except ImportError:
        from jax import shard_map
    from concourse import bass2jax
    from concourse.bass2jax import _bass_exec_p, partition_id_tensor

    bass2jax.install_neuronx_cc_hook()
    n_cores = NCORES
    pname = nc.partition_id_tensor.name if nc.partition_id_tensor else None
    in_names, out_names, out_avals, zero_outs = [], [], [], []
    for alloc in nc.m.functions[0].allocations:
        if not isinstance(alloc, mybir.MemoryLocationSet):
            continue
        name = alloc.memorylocations[0].name
        if alloc.kind == "ExternalInput":
            if name != pname:
                in_names.append(name)
        elif alloc.kind == "ExternalOutput":
            shape = tuple(alloc.tensor_shape)
            dtype = mybir.dt.np(alloc.dtype)
            out_avals.append(jax.core.ShapedArray(shape, dtype))
            zero_outs.append(np.zeros(shape, dtype))
            out_names.append(name)
    n_params = len(in_names)
    n_outs = len(out_avals)
    in_names_full = in_names + out_names + ([pname] if pname else [])

    def _body(*args):
        operands = list(args)
        if pname is not None:
            operands.append(partition_id_tensor())
        outs = _bass_exec_p.bind(
            *operands, out_avals=tuple(out_avals),
            in_names=tuple(in_names_full), out_names=tuple(out_names),
            lowering_input_output_aliases=(), sim_require_finite=True,
            sim_require_nnan=True, nc=nc)
        return tuple(outs)

    devices = jax.devices()[:n_cores]
    mesh = Mesh(np.asarray(devices), ("core",))
    in_specs = (PartitionSpec("core"),) * (n_params + n_outs)
    out_specs = (PartitionSpec("core"),) * len(out_names)
    donate = tuple(range(n_params, n_params + n_outs))
    sharded = jax.jit(shard_map(_body, mesh=mesh, in_specs=in_specs,
                                out_specs=out_specs, check_rep=False),
                      donate_argnums=donate, keep_unused=True)
    per_core = [[np.asarray(m[name]) for name in in_names] for m in in_maps]
    concat_in = [np.concatenate([per_core[c][i] for c in range(n_cores)],
                                axis=0) for i in range(n_params)]
    compiled = sharded.lower(
        *concat_in,
        *[np.zeros((n_cores * z.shape[0], *z.shape[1:]), z.dtype)
          for z in zero_outs]).compile()
    sh = NamedSharding(mesh, PartitionSpec("core"))
    dev_in = [jax.device_put(a, sh) for a in concat_in]
    jax.block_until_ready(dev_in)
    return dict(compiled=compiled, dev_in=dev_in, zero_outs=zero_outs,
                out_names=out_names, out_avals=out_avals, sh=sh,
                n_cores=n_cores)


def fresh_zeros(B):
    """Donated output buffers (device-resident) for one execution."""
    import jax
    zs = [jax.device_put(
        np.zeros((B["n_cores"] * z.shape[0], *z.shape[1:]), z.dtype), B["sh"])
        for z in B["zero_outs"]]
    jax.block_until_ready(zs)
    return zs


def run_exec(B, zs=None):
    """One kernel execution; returns per-core output dicts."""
    import jax
    if zs is None:
        zs = fresh_zeros(B)
    outs = B["compiled"](*B["dev_in"], *zs)
    jax.block_until_ready(outs)
    return [
        {name: np.asarray(outs[i]).reshape(B["n_cores"],
                                           *B["out_avals"][i].shape)[c]
         for i, name in enumerate(B["out_names"])}
        for c in range(B["n_cores"])
    ]


_CACHE = {}


def kernel(**inputs):
    key = tuple(sorted((k, tuple(np.asarray(v).shape))
                       for k, v in inputs.items()))
    ent = _CACHE.get(key)
    digest = None
    try:
        import hashlib
        h = hashlib.sha1()
        for k in sorted(inputs):
            h.update(np.ascontiguousarray(np.asarray(inputs[k])).tobytes())
        digest = h.hexdigest()
    except Exception:
        pass
    if ent is None or ent["digest"] != digest or digest is None:
        nc, in_maps, assemble = prepare(**inputs)
        try:
            B = build_exec(nc, in_maps)
        except Exception:
            B = None
        ent = {"nc": nc, "in_maps": in_maps, "assemble": assemble, "B": B,
               "digest": digest}
        _CACHE[key] = ent
    globals()["LAST_RUN"] = {
        "nc": ent["nc"], "in_maps": ent["in_maps"], "exec_time_ns": None,
        "B": ent["B"],
    }
    if ent["B"] is not None:
        outs = run_exec(ent["B"])
        return ent["assemble"]([outs[c]["outT"] for c in range(NCORES)])
    res = run_bass_kernel_spmd(ent["nc"], ent["in_maps"],
                               core_ids=list(range(NCORES)))
    return ent["assemble"]([res.results[c]["outT"] for c in range(NCORES)])


# revision 49
# speedup vs baseline: 1.0135x; 1.0024x over previous
"""DeepSAT GNN message-passing kernel for 8 TRN2 NeuronCores.

Mathematical restructuring exploited (validated vs reference to ~1e-7):
  * Every node updates exactly once, at iteration l == forward_level[node],
    always starting from the same constant hidden state h_init (broadcast of
    emd_w[:,0]+emd_b).  Hence gh = w_hh @ h_init + b_hh is one constant vector.
  * Sources that have not updated yet (src_level == 0 or src_level >= tgt_level)
    contribute the constant agg(h_init) to the aggregation -> those edges fold
    into per-target integer counts (host-side index work).
  * The aggregation transform is linear, so msg @ w_ih_m.T fuses into
    s @ (w_ih_m @ agg_w).T  where s = sum of raw h[src] over active edges.
  * The per-target affine terms (counts, degree, bias) ride along as 3 extra
    rows of a K=19 matmul operand together with x.T.
  * The 3-layer MLP head is fused into the level loop (h tiles are consumed
    feature-major straight out of the GRU), so the last level needs no
    AllGather and no h table entry.

Distribution (graph parallel, SPMD — one program, per-core data via inputs):
  * Nodes renumbered by (level, owning core, position); each level's nodes
    split into 8 equal chunks, one per core.
  * Replicated h tables split into window tensors of <= 32767 rows (2 levels
    each) so rows are addressable by int16 dma_gather indices.
  * Edges sharded by target core, grouped by (target level, source window);
    one dma_gather per group; segment-sum via one-hot matmuls into PSUM.
  * After each level, updated h rows are AllGather'd into the level's window
    tensor slice.
  * Gathers (and their segment-sum matmuls) for windows finalized 2+ levels
    ago are issued on the Pool queue BEFORE the previous level's AllGather,
    so they overlap it; only the newest window's gather waits.
"""

import os

import numpy as np

import concourse.bacc as bacc
import concourse.bass as bass
import concourse.mybir as mybir
import concourse.tile as tile
from concourse import library_config
from concourse.bass_utils import run_bass_kernel_spmd
from concourse.tile_rust import add_dep_helper

NCORES = 8
TILE_T = 512   # targets per compute tile (one PSUM bank of fp32)
ECHUNK = 128   # edges per segsum matmul chunk (K partition dim)
WIN_ROWS = 32767  # int16 dma_gather index limit per window tensor

F32 = mybir.dt.float32
H_DT = mybir.dt.float16   # h storage + compute dtype (table, gather, gates)
H_NP = np.float16

PAD_TOFF = 65504.0  # one-hot scalar for inactive edges: never equals iota


def _host_prep(x, edge_index, forward_level, L):
    """Integer/index preprocessing: renumbering, windows, edge groups,
    per-core input arrays."""
    N, DX = x.shape
    lvl = np.asarray(forward_level).astype(np.int64)
    src = np.asarray(edge_index[0]).astype(np.int64)
    tgt = np.asarray(edge_index[1]).astype(np.int64)

    sl, tl = lvl[src], lvl[tgt]
    active = (sl >= 1) & (sl < tl) & (tl < L)
    const_e = (~active) & (tl >= 1) & (tl < L)

    deg = np.bincount(tgt, minlength=N).astype(np.float32)
    cnt = np.bincount(tgt[const_e], minlength=N).astype(np.float32)

    # nodes whose h is ever gathered (>=1 active out-edge); only they need
    # to be staged / AllGather'd / kept in window tables
    used = np.zeros(N, bool)
    used[np.unique(src[active])] = True

    core_of = np.zeros(N, np.int64)
    pos_of = np.zeros(N, np.int64)
    chunk = [0] * L
    stage_rows = [0] * L   # rows staged+AG'd per core per level (used prefix)
    nodes_by_level = []
    for l in range(L):
        nodes = np.flatnonzero(lvl == l)
        nodes_by_level.append(nodes)
        if l >= 1 and len(nodes):
            chunk[l] = (len(nodes) + NCORES - 1) // NCORES
            mx_used = 0
            for c in range(NCORES):
                seg = nodes[c * chunk[l]:(c + 1) * chunk[l]]
                u = used[seg]
                order = np.argsort(~u, kind="stable")
                core_of[seg] = c
                pos_of[seg[order]] = np.arange(len(seg))
                mx_used = max(mx_used, int(u.sum()))
            if 1 <= l <= L - 2:
                stage_rows[l] = mx_used

    # ---- window tensors over source levels 1..L-2 ----
    # few levels per window: every window except the newest is final before
    # the previous level's AllGather, so its gathers + segment-sums can be
    # issued ahead of the AG and overlap it; fewer levels per window =
    # more prefetchable edges but more (padded) gather groups
    WPL = int(os.environ.get("KWPL", "0"))
    win_of = [-1] * L        # level -> window index
    lvl_off = [0] * L        # level -> row offset inside its window
    win_rows = []            # window -> total rows
    win_nlvl = 0
    for l in range(1, L - 1):
        rl = NCORES * stage_rows[l]
        assert rl <= WIN_ROWS, "one level exceeds the int16 window"
        if (not win_rows or win_rows[-1] + rl > WIN_ROWS
                or (WPL and win_nlvl >= WPL)):
            win_rows.append(0)
            win_nlvl = 0
        win_of[l] = len(win_rows) - 1
        lvl_off[l] = win_rows[-1]
        win_rows[-1] += rl
        win_nlvl += 1
    NWIN = len(win_rows)

    # per-core staging rows for AG inputs (levels 1..L-2, local order)
    stg_start = [0] * L
    sr = 0
    for l in range(1, L - 1):
        stg_start[l] = sr
        sr += stage_rows[l]
    R_stg = max(sr, 1)

    # per-core owned output columns (levels 1..L-1)
    own_start = [0] * L
    oc = 0
    for l in range(1, L):
        own_start[l] = oc
        oc += chunk[l]
    OWN = max(oc, 1)

    ntiles = [0] * L
    for l in range(1, L):
        ntiles[l] = (chunk[l] + TILE_T - 1) // TILE_T if chunk[l] else 0

    # xaux [core, DX+3, OWN]: x.T rows, then cnt, deg, ones
    xaux = np.zeros((NCORES, DX + 3, OWN), H_NP)
    for l in range(1, L):
        nodes = nodes_by_level[l]
        if not len(nodes):
            continue
        c, p = core_of[nodes], pos_of[nodes]
        cols = own_start[l] + p
        xaux[c, :DX, cols] = np.asarray(x)[nodes].astype(H_NP)
        xaux[c, DX, cols] = cnt[nodes]
        xaux[c, DX + 1, cols] = deg[nodes]
        xaux[c, DX + 2, cols] = 1.0

    # ---- active edges -> (core, tgt level, src window) groups ----
    a_src, a_tgt = src[active], tgt[active]
    e_widx = (np.array(lvl_off)[lvl[a_src]]
              + core_of[a_src] * np.array(stage_rows)[lvl[a_src]]
              + pos_of[a_src]).astype(np.int64)
    # group axis g = 2*window + fresh: "fresh" edges (source exactly one
    # level below the target) are the ONLY ones that need the immediately
    # preceding AllGather; older-source groups in the same window tensor
    # can gather + segment-sum ahead of it
    e_w = np.array(win_of)[lvl[a_src]]
    e_fresh = (lvl[a_src] == lvl[a_tgt] - 1).astype(np.int64)
    e_g = e_w * 2 + e_fresh
    NG = 2 * NWIN
    e_core = core_of[a_tgt]
    e_lvl = lvl[a_tgt]
    e_tile = pos_of[a_tgt] // TILE_T
    e_toff = (pos_of[a_tgt] % TILE_T).astype(np.float32)

    order = np.lexsort((e_toff, e_tile, e_g, e_lvl, e_core))
    e_widx, e_g, e_core, e_lvl, e_tile, e_toff = (
        a[order] for a in (e_widx, e_g, e_core, e_lvl, e_tile, e_toff))

    # split points per (core, level, group)
    per = {}
    key = (e_core * L + e_lvl) * NG + e_g
    uk, ustart, ucnt = np.unique(key, return_index=True, return_counts=True)
    for k, s0, n in zip(uk, ustart, ucnt):
        g = int(k % NG)
        l = int((k // NG) % L)
        c = int(k // (NG * L))
        per[(c, l, g)] = (int(s0), int(n))

    # group schedule: (l, g) -> padded size, chunk count
    groups = {}   # (l, g) -> dict(num, nch, gcol, rows_avail)
    IDXCOLS = 0
    rows_written = [0] * NWIN   # rows present in window w before level l
    rows_hist = {}              # (l, w) -> rows readable at level l
    for l in range(1, L):
        for w in range(NWIN):
            rows_hist[(l, w)] = rows_written[w]
        if 1 <= l <= L - 2 and chunk[l]:
            rows_written[win_of[l]] += NCORES * stage_rows[l]
    for l in range(2, L):
        for g in range(NG):
            mx = max((per.get((c, l, g), (0, 0))[1] for c in range(NCORES)),
                     default=0)
            if mx == 0:
                continue
            nch = (mx + ECHUNK - 1) // ECHUNK
            w = g // 2
            # old groups (g even) only reference rows final before level l-1
            rows = rows_hist[(l, w)] if g % 2 else rows_hist[(l - 1, w)]
            groups[(l, g)] = dict(num=nch * ECHUNK, nch=nch, gcol=IDXCOLS,
                                  rows=rows)
            IDXCOLS += (nch * ECHUNK) // 16
    IDXCOLS = max(IDXCOLS, 1)

    # occurrences: (l, g, chunk k, tile t) present on any core
    occ_set = set()
    for (c, l, g), (s0, n) in per.items():
        tiles_of = e_tile[s0:s0 + n]
        for k in range(groups[(l, g)]["nch"]):
            a, b = k * ECHUNK, min((k + 1) * ECHUNK, n)
            if a >= n:
                break
            for t in np.unique(tiles_of[a:b]):
                occ_set.add((l, int(g), k, int(t)))
    occs = sorted(occ_set)
    occ_col = {o: i for i, o in enumerate(occs)}
    NOCC = max(len(occs), 1)
    occs_by_tile = {}
    for (l, g, k, t) in occs:
        occs_by_tile.setdefault((l, t), []).append((g, k, occ_col[(l, g, k, t)]))

    # per-core arrays
    gidx16 = np.zeros((NCORES, 128, IDXCOLS), np.int16)
    tofff = np.full((NCORES, 128, NOCC), PAD_TOFF, np.float32)
    for (c, l, gk), (s0, n) in per.items():
        g = groups[(l, gk)]
        num, nch, gcol = g["num"], g["nch"], g["gcol"]
        idxs = np.zeros(num, np.int16)
        idxs[:n] = e_widx[s0:s0 + n].astype(np.int16)
        wrapped = idxs.reshape(num // 16, 16).T  # [16, num/16]
        gidx16[c, :, gcol:gcol + num // 16] = np.tile(wrapped, (8, 1))
        tiles_of = e_tile[s0:s0 + n]
        toffs_of = e_toff[s0:s0 + n]
        for k in range(nch):
            a = k * ECHUNK
            b = min(a + ECHUNK, n)
            if a >= n:
                break
            for t in np.unique(tiles_of[a:b]):
                col = occ_col[(l, gk, k, t)]
                seg = np.full(ECHUNK, PAD_TOFF, np.float32)
                m = tiles_of[a:b] == t
                seg[:b - a][m] = toffs_of[a:b][m]
                tofff[c, :, col] = seg

    return dict(
        N=N, DX=DX, lvl=lvl, chunk=chunk, ntiles=ntiles,
        stage_rows=stage_rows,
        win_of=win_of, lvl_off=lvl_off, win_rows=win_rows, NWIN=NWIN, NG=NG,
        groups=groups, occs_by_tile=occs_by_tile,
        IDXCOLS=IDXCOLS, NOCC=NOCC, R_stg=R_stg,
        stg_start=stg_start, own_start=own_start, OWN=OWN,
        core_of=core_of, pos_of=pos_of,
        gidx16=gidx16, tofff=tofff, xaux=xaux,
    )


def _build_program(prep, consts, L, DH, DM):
    """Build the SPMD Bass program (identical across cores)."""
    DX = prep["DX"]
    OWN, IDXCOLS, NOCC = prep["OWN"], prep["IDXCOLS"], prep["NOCC"]
    ntiles, chunk = prep["ntiles"], prep["chunk"]
    stage_rows = prep["stage_rows"]
    groups, occs_by_tile = prep["groups"], prep["occs_by_tile"]
    win_of, lvl_off, win_rows = prep["win_of"], prep["lvl_off"], prep["win_rows"]
    stg_start, own_start = prep["stg_start"], prep["own_start"]
    R_stg = prep["R_stg"]
    NWIN = prep["NWIN"]
    NG = prep["NG"]
    OUT_COLS = 1 + OWN
    G3 = 3 * DH

    nc = bacc.Bacc("TRN2", target_bir_lowering=False, debug=False)

    gidx_t = nc.dram_tensor("gidx", [128, IDXCOLS], mybir.dt.int16,
                            kind="ExternalInput")
    toff_t = nc.dram_tensor("toff", [128, NOCC], F32, kind="ExternalInput")
    xaux_t = nc.dram_tensor("xaux", [DX + 3, OWN], H_DT, kind="ExternalInput")
    outT = nc.dram_tensor("outT", [1, OUT_COLS], F32, kind="ExternalOutput")

    wtbl = [nc.dram_tensor(f"wtbl{w}", [max(r, 1), DH], H_DT,
                           addr_space="Shared")
            for w, r in enumerate(win_rows)]
    h_stage = nc.dram_tensor("h_stage", [R_stg, DH], H_DT)

    WcT_c = nc.inline_tensor(consts["WcT"], "WcT")
    WxT_c = nc.inline_tensor(consts["WxT"], "WxT")
    ghnd_c = nc.inline_tensor(consts["ghnd"], "ghnd")
    hinit_c = nc.inline_tensor(consts["hinit"], "hinit")
    hinit32_c = nc.inline_tensor(consts["hinit32"], "hinit32")
    W1T_c = nc.inline_tensor(consts["W1T"], "W1T")
    W2T_c = nc.inline_tensor(consts["W2T"], "W2T")
    W3T_c = nc.inline_tensor(consts["W3T"], "W3T")
    b1_c = nc.inline_tensor(consts["b1"], "b1")
    b2_c = nc.inline_tensor(consts["b2"], "b2")
    iota_c = nc.inline_tensor(
        np.tile(np.arange(TILE_T, dtype=H_NP), (128, 1)), "iota")
    ident_c = nc.inline_tensor(np.eye(128, dtype=H_NP), "ident")
    b3f = float(consts["b3"])
    RG = [list(range(NCORES))]
    AF = mybir.ActivationFunctionType
    ALU = mybir.AluOpType

    with tile.TileContext(nc, num_cores=NCORES) as tc:
        with tc.tile_pool(name="cst", bufs=1) as cst, \
             tc.tile_pool(name="gat", bufs=int(os.environ.get("KGAT", "16"))) as gat, \
             tc.tile_pool(name="wrk", bufs=2) as wrk, \
             tc.tile_pool(name="acc", bufs=5) as acc, \
             tc.tile_pool(name="hp", bufs=5) as hp, \
             tc.tile_pool(name="psA", bufs=2, space="PSUM") as psA, \
             tc.tile_pool(name="ps", bufs=1, space="PSUM") as ps:

            nc.gpsimd.load_library(library_config.mlp)

            # ---- constants to SBUF ----
            def cload(name, src, shape, dtype=F32):
                t = cst.tile(shape, dtype, tag=name)
                nc.sync.dma_start(out=t[:], in_=src[:, :])
                return t

            iota = cload("iota", iota_c, [128, TILE_T], H_DT)
            ident = cload("ident", ident_c, [128, 128], H_DT)
            WcT = cload("WcT", WcT_c, [DH, G3], H_DT)
            WxT = cload("WxT", WxT_c, [DX + 3, G3], H_DT)
            ghnd = cload("ghnd", ghnd_c, [DH, DH], H_DT)
            hinit = cload("hinit", hinit_c, [DH, 1], H_DT)
            hinit32 = cload("hinit32", hinit32_c, [DH, 1])
            W1T = cload("W1T", W1T_c, [DH, DM], H_DT)
            W2T = cload("W2T", W2T_c, [DM, DM], H_DT)
            W3T = cload("W3T", W3T_c, [DM, 1], H_DT)
            b1 = cload("b1", b1_c, [DM, 1])
            b2 = cload("b2", b2_c, [DM, 1])
            gidx_sb = cload("gidx", gidx_t, [128, IDXCOLS], mybir.dt.int16)
            toff_sb = cload("toff", toff_t, [128, NOCC])
            xauxsb = cload("xauxsb", xaux_t, [DX + 3, OWN], H_DT)
            obuf = cst.tile([1, OUT_COLS], F32, tag="obuf")

            RELU_DVE = os.environ.get("KRELU", "act") == "dve"
            OUT_DVE = os.environ.get("KOUT", "act") == "dve"

            def mlp(hT_sb, n_t, out_col):
                # z1 / z2 / out share one PSUM bank on disjoint partitions;
                # results accumulate in obuf (one outT DMA at the end).
                mp = ps.tile([2 * DM + 1, TILE_T], F32, tag="mlp")
                nc.tensor.matmul(mp[0:DM, :n_t], lhsT=W1T[:], rhs=hT_sb,
                                 start=True, stop=True)
                z1s = wrk.tile([DM, TILE_T], H_DT, tag="z1s")
                if RELU_DVE:
                    nc.vector.tensor_scalar(out=z1s[:, :n_t],
                                            in0=mp[0:DM, :n_t],
                                            scalar1=b1[:, 0:1], scalar2=0.0,
                                            op0=ALU.add, op1=ALU.max)
                else:
                    nc.scalar.activation(out=z1s[:, :n_t], in_=mp[0:DM, :n_t],
                                         func=AF.Relu, bias=b1[:, 0:1])
                nc.tensor.matmul(mp[DM:2 * DM, :n_t], lhsT=W2T[:],
                                 rhs=z1s[:, :n_t], start=True, stop=True)
                z2s = wrk.tile([DM, TILE_T], H_DT, tag="z2s")
                if RELU_DVE:
                    nc.vector.tensor_scalar(out=z2s[:, :n_t],
                                            in0=mp[DM:2 * DM, :n_t],
                                            scalar1=b2[:, 0:1], scalar2=0.0,
                                            op0=ALU.add, op1=ALU.max)
                else:
                    nc.scalar.activation(out=z2s[:, :n_t],
                                         in_=mp[DM:2 * DM, :n_t],
                                         func=AF.Relu, bias=b2[:, 0:1])
                nc.tensor.matmul(mp[2 * DM:2 * DM + 1, :n_t], lhsT=W3T[:],
                                 rhs=z2s[:, :n_t], start=True, stop=True)
                if OUT_DVE:
                    nc.vector.tensor_scalar(
                        out=obuf[0:1, out_col:out_col + n_t],
                        in0=mp[2 * DM:2 * DM + 1, :n_t],
                        scalar1=b3f, scalar2=None, op0=ALU.add)
                else:
                    nc.scalar.activation(out=obuf[0:1, out_col:out_col + n_t],
                                         in_=mp[2 * DM:2 * DM + 1, :n_t],
                                         func=AF.Copy, bias=b3f)

            # output column 0: MLP(h_init) for never-updated nodes
            mlp(hinit[:, 0:1], 1, 0)

            KREPS = int(os.environ.get("KREPS", "1"))
            STAGE_MERGE = os.environ.get("KSTAGE", "merge") == "merge"
            STAGE_PSUM = os.environ.get("KSTAGE", "merge") == "psum"
            MLP_DEFER = os.environ.get("KMLPDEF", "1") == "1"

            # Early-ready producers (gathers on old windows, one-hots) must
            # acquire pool slots roughly in program order or the scheduler's
            # slot waits can form cycles.  Anchor them to recent per-tile
            # instructions.
            anchors = []   # one per processed tile: the sigmoid activation
            gchain = []
            GCHAIN = int(os.environ.get("KGCH", "4"))
            gtiles = {}    # (l, w) -> (gather tile, gather ins)

            def emit_gather(l, g):
                gr = groups[(l, g)]
                num, nch, gcol = gr["num"], gr["nch"], gr["gcol"]
                gt = gat.tile([128, nch * DH], H_DT, tag="g")
                gi = nc.gpsimd.dma_gather(
                    gt[:].rearrange("p (q e) -> p q e", e=DH),
                    wtbl[g // 2][0:gr["rows"], :],
                    gidx_sb[:, gcol:gcol + num // 16],
                    num, num, DH)
                gchain.append(gi.ins)
                if len(gchain) > GCHAIN:
                    add_dep_helper(gchain[-1], gchain[-1 - GCHAIN],
                                   sync=True, reason="gather slot pacing")
                if anchors:
                    add_dep_helper(gi.ins, anchors[-1], sync=True,
                                   reason="gather level pacing")
                gtiles[(l, g)] = (gt, gi.ins)

            for rep in range(KREPS):
             for l in range(1, L):
                # gathers not issued during the previous level (fresh groups
                # need AG(l-1); everything at a rep's first levels)
                for g in range(NG):
                    if (l, g) in groups and (l, g) not in gtiles:
                        emit_gather(l, g)

                def occ_mms(occ_list, n_t, l):
                    sTp = psA.tile([DH, TILE_T], F32, tag="sT")
                    for i, (g, k, col) in enumerate(occ_list):
                        oh = wrk.tile([ECHUNK, TILE_T], H_DT, tag="oh")
                        ohi = nc.vector.tensor_scalar(
                            out=oh[:, :n_t], in0=iota[:, :n_t],
                            scalar1=toff_sb[:, col:col + 1], scalar2=None,
                            op0=ALU.is_equal)
                        add_dep_helper(ohi.ins, gtiles[(l, g)][1],
                                       sync=True, reason="onehot pacing")
                        nc.tensor.matmul(
                            sTp[:, :n_t],
                            lhsT=gtiles[(l, g)][0][:, k * DH:(k + 1) * DH],
                            rhs=oh[:, :n_t],
                            start=(i == 0), stop=(i == len(occ_list) - 1))
                    return sTp

                # PASS A: old-group segment-sums for ALL tiles first (only
                # fresh groups — source level == l-1 — depend on AG(l-1);
                # old groups overlap it.  Emitting any fresh one-hot earlier
                # would head-of-line-block the DVE queue.
                sAcc = {}
                for t in range(ntiles[l]):
                    n_t = min(TILE_T, chunk[l] - t * TILE_T)
                    old = [o for o in occs_by_tile.get((l, t), [])
                           if o[0] % 2 == 0]
                    if old:
                        sTp = occ_mms(old, n_t, l)
                        a = acc.tile([DH, TILE_T], H_DT, tag="sAcc")
                        nc.vector.tensor_copy(out=a[:, :n_t],
                                              in_=sTp[:, :n_t])
                        sAcc[t] = a

                # PASS B: newest-window segment-sums + GRU per tile
                pend_mlp = []
                for t in range(ntiles[l]):
                    n_t = min(TILE_T, chunk[l] - t * TILE_T)
                    new = [o for o in occs_by_tile.get((l, t), [])
                           if o[0] % 2 == 1]
                    sT_sb = None
                    if new:
                        sTp = occ_mms(new, n_t, l)
                        sT_sb = wrk.tile([DH, TILE_T], H_DT, tag="sTs")
                        if t in sAcc:
                            nc.vector.tensor_tensor(
                                out=sT_sb[:, :n_t], in0=sTp[:, :n_t],
                                in1=sAcc[t][:, :n_t], op=ALU.add)
                        else:
                            nc.vector.tensor_copy(out=sT_sb[:, :n_t],
                                                  in_=sTp[:, :n_t])
                    elif t in sAcc:
                        sT_sb = sAcc[t]

                    oc0 = own_start[l] + t * TILE_T
                    xa = xauxsb[:, oc0:oc0 + n_t]

                    # r gate at cols [0:n_t], z gate bank-aligned at
                    # [TILE_T : TILE_T+n_t] (a matmul must not cross banks)
                    # dep-free WxT@xa accumulates FIRST so the in-order PE
                    # starts each tile's gates while DVE still merges sT
                    girz = ps.tile([DH, 2 * TILE_T], F32, tag="girz")
                    for gi_g in range(2):
                        gsl = slice(gi_g * TILE_T, gi_g * TILE_T + n_t)
                        wsl = slice(gi_g * DH, (gi_g + 1) * DH)
                        nc.tensor.matmul(girz[:, gsl], lhsT=WxT[:, wsl],
                                         rhs=xa,
                                         start=True, stop=(sT_sb is None))
                        if sT_sb is not None:
                            nc.tensor.matmul(girz[:, gsl], lhsT=WcT[:, wsl],
                                             rhs=sT_sb[:, :n_t],
                                             start=False, stop=True)
                    gin = ps.tile([DH, TILE_T], F32, tag="gin")
                    nc.tensor.matmul(gin[:, :n_t], lhsT=WxT[:, 2 * DH:G3],
                                     rhs=xa,
                                     start=True, stop=False)
                    if sT_sb is not None:
                        nc.tensor.matmul(gin[:, :n_t], lhsT=WcT[:, 2 * DH:G3],
                                         rhs=sT_sb[:, :n_t],
                                         start=False, stop=False)

                    rz = wrk.tile([DH, 2 * TILE_T], H_DT, tag="rz")
                    if n_t == TILE_T:
                        sgi = nc.scalar.activation(out=rz[:, :2 * TILE_T],
                                                   in_=girz[:, :2 * TILE_T],
                                                   func=AF.Sigmoid)
                    else:
                        nc.scalar.activation(
                            out=rz[:, TILE_T:TILE_T + n_t],
                            in_=girz[:, TILE_T:TILE_T + n_t], func=AF.Sigmoid)
                        sgi = nc.scalar.activation(out=rz[:, :n_t],
                                                   in_=girz[:, :n_t],
                                                   func=AF.Sigmoid)
                    anchors.append(sgi.ins)
                    nc.tensor.matmul(gin[:, :n_t], lhsT=ghnd[:],
                                     rhs=rz[:, :n_t], start=False, stop=True)
                    n_sb = wrk.tile([DH, TILE_T], H_DT, tag="n")
                    nc.scalar.activation(out=n_sb[:, :n_t], in_=gin[:, :n_t],
                                         func=AF.Tanh)
                    t3 = wrk.tile([DH, TILE_T], H_DT, tag="t3")
                    nc.vector.tensor_scalar(out=t3[:, :n_t], in0=n_sb[:, :n_t],
                                            scalar1=hinit32[:, 0:1], scalar2=None,
                                            op0=ALU.subtract)
                    t4 = wrk.tile([DH, TILE_T], H_DT, tag="t4")
                    nc.vector.tensor_tensor(out=t4[:, :n_t],
                                            in0=rz[:, TILE_T:TILE_T + n_t],
                                            in1=t3[:, :n_t],
                                            op=ALU.mult)
                    hT = hp.tile([DH, TILE_T], H_DT, tag="hT")
                    nc.vector.tensor_tensor(out=hT[:, :n_t], in0=n_sb[:, :n_t],
                                            in1=t4[:, :n_t],
                                            op=ALU.subtract)

                    # MLP head deferred past the AllGather: it has no
                    # downstream consumer until the final outT DMA, and
                    # emitting it here would delay the next tile's gate
                    # matmuls/activations on the in-order PE/Act queues
                    if MLP_DEFER:
                        pend_mlp.append((hT, n_t, 1 + oc0))
                    else:
                        mlp(hT[:, :n_t], n_t, 1 + oc0)

                    # stage only the used prefix (nodes some edge gathers)
                    stage_n = min(n_t, max(0, stage_rows[l] - t * TILE_T))
                    if l <= L - 2 and stage_n > 0:
                        trp = ps.tile([128, TILE_T], H_DT, tag="tr")
                        nch_t = (stage_n + 127) // 128
                        st = wrk.tile([128, TILE_T], H_DT, tag="st")
                        for ci in range(nch_t):
                            wdt = min(128, stage_n - ci * 128)
                            nc.tensor.transpose(
                                out=trp[:wdt, ci * 128:ci * 128 + 128],
                                in_=hT[:, ci * 128:ci * 128 + wdt],
                                identity=ident[:])
                        r0 = stg_start[l] + t * TILE_T
                        if STAGE_PSUM:
                            # DMA straight out of the PSUM transpose tile:
                            # removes the DVE copy from the pre-AG chain
                            if stage_n == TILE_T:
                                nc.sync.dma_start(
                                    out=h_stage[r0:r0 + TILE_T, :].rearrange(
                                        "(q p) e -> p q e", p=128),
                                    in_=trp[:].rearrange("p (q e) -> p q e",
                                                         e=DH))
                            else:
                                for ci in range(nch_t):
                                    wdt = min(128, stage_n - ci * 128)
                                    nc.sync.dma_start(
                                        out=h_stage[r0 + ci * 128:
                                                    r0 + ci * 128 + wdt, :],
                                        in_=trp[:wdt,
                                                ci * 128:ci * 128 + 128])
                        elif STAGE_MERGE and stage_n == TILE_T:
                            nc.vector.tensor_copy(out=st[:, :TILE_T],
                                                  in_=trp[:, :TILE_T])
                            nc.sync.dma_start(
                                out=h_stage[r0:r0 + TILE_T, :].rearrange(
                                    "(q p) e -> p q e", p=128),
                                in_=st[:].rearrange("p (q e) -> p q e", e=DH))
                        else:
                            for ci in range(nch_t):
                                wdt = min(128, stage_n - ci * 128)
                                nc.vector.tensor_copy(
                                    out=st[:wdt, ci * 128:ci * 128 + 128],
                                    in_=trp[:wdt, ci * 128:ci * 128 + 128])
                                nc.sync.dma_start(
                                    out=h_stage[r0 + ci * 128:
                                                r0 + ci * 128 + wdt, :],
                                    in_=st[:wdt, ci * 128:ci * 128 + 128])

                # pre-issue next level's old-group gathers (sources at
                # levels <= l-1, already final): they overlap the AG on the
                # Pool queue; fresh groups wait until level l+1
                if l + 1 < L:
                    for g in range(0, NG, 2):
                        if (l + 1, g) in groups:
                            emit_gather(l + 1, g)

                if l <= L - 2 and stage_rows[l] > 0:
                    w = win_of[l]
                    o0 = lvl_off[l]
                    sr_l = stage_rows[l]
                    if os.environ.get("KSKIP_AG"):
                        # timing-skeleton mode: local copy instead of AG
                        # (results are wrong across cores; sim feedback only)
                        nc.sync.dma_start(
                            out=wtbl[w][o0:o0 + sr_l, :],
                            in_=h_stage[stg_start[l]:stg_start[l] + sr_l, :])
                    else:
                        nc.gpsimd.collective_compute(
                            "AllGather", mybir.AluOpType.bypass,
                            replica_groups=RG,
                            ins=[h_stage[stg_start[l]:stg_start[l] + sr_l, :]],
                            outs=[wtbl[w][o0:o0 + NCORES * sr_l, :]],
                        )

                # deferred MLP heads: fill the PE/Act/DVE queues while the
                # AllGather runs on the Pool queue
                for hT_p, n_t_p, col_p in pend_mlp:
                    mlp(hT_p[:, :n_t_p], n_t_p, col_p)

                # this level's gather tiles are consumed; drop the refs
                for key in [k for k in gtiles if k[0] == l]:
                    del gtiles[key]

             # end of rep: final outT DMA emitted once, after the last rep
            nc.sync.dma_start(out=outT[0:1, :], in_=obuf[:, :])

    nc.compile()
    return nc


def _make_consts(DX, DH, emd_w, emd_b, agg_w, agg_b, w_ih, w_hh, b_ih, b_hh,
                 mlp_w1, mlp_b1, mlp_w2, mlp_b2, mlp_w3, mlp_b3):
    h_init = emd_w[:, 0] + emd_b
    gh = w_hh @ h_init + b_hh
    Wm, Wx = w_ih[:, :DH], w_ih[:, DH:]
    W_comb = Wm @ agg_w
    u1 = Wm @ (agg_w @ h_init)
    u2 = Wm @ agg_b
    cb = b_ih.copy()
    cb[:2 * DH] += gh[:2 * DH]
    WxauxT = np.zeros((DX + 3, 3 * DH), np.float32)
    WxauxT[:DX] = Wx.T
    WxauxT[DX] = u1
    WxauxT[DX + 1] = u2
    WxauxT[DX + 2] = cb
    return dict(
        WcT=np.ascontiguousarray(W_comb.T).astype(H_NP),
        WxT=np.ascontiguousarray(WxauxT).astype(H_NP),
        ghnd=np.diag(gh[2 * DH:]).astype(H_NP),
        hinit=h_init[:, None].astype(H_NP),
        hinit32=h_init[:, None].astype(np.float32),
        W1T=np.ascontiguousarray(mlp_w1.T).astype(H_NP),
        W2T=np.ascontiguousarray(mlp_w2.T).astype(H_NP),
        W3T=np.ascontiguousarray(mlp_w3.T).astype(H_NP),
        b1=mlp_b1[:, None].astype(np.float32),
        b2=mlp_b2[:, None].astype(np.float32),
        b3=mlp_b3.reshape(-1)[0],
    )


def prepare(x, edge_index, forward_level, num_layers_f,
            emd_w, emd_b, agg_w, agg_b, w_ih, w_hh, b_ih, b_hh,
            mlp_w1, mlp_b1, mlp_w2, mlp_b2, mlp_w3, mlp_b3):
    """Host prep + program build; returns (nc, in_maps, assemble)."""
    x = np.asarray(x, np.float32)
    L = int(np.asarray(num_layers_f))
    N, DX = x.shape
    DH = np.asarray(agg_w).shape[0]
    DM = np.asarray(mlp_w1).shape[0]
    consts = _make_consts(
        DX, DH,
        np.asarray(emd_w, np.float32), np.asarray(emd_b, np.float32),
        np.asarray(agg_w, np.float32), np.asarray(agg_b, np.float32),
        np.asarray(w_ih, np.float32), np.asarray(w_hh, np.float32),
        np.asarray(b_ih, np.float32), np.asarray(b_hh, np.float32),
        np.asarray(mlp_w1, np.float32), np.asarray(mlp_b1, np.float32),
        np.asarray(mlp_w2, np.float32), np.asarray(mlp_b2, np.float32),
        np.asarray(mlp_w3, np.float32), np.asarray(mlp_b3, np.float32))

    prep = _host_prep(x, np.asarray(edge_index), forward_level, L)
    nc = _build_program(prep, consts, L, DH, DM)
    in_maps = [
        {"gidx": np.ascontiguousarray(prep["gidx16"][c]),
         "toff": np.ascontiguousarray(prep["tofff"][c]),
         "xaux": np.ascontiguousarray(prep["xaux"][c])}
        for c in range(NCORES)
    ]

    def assemble(outs):
        """outs: [NCORES] arrays of shape [1, OUT_COLS] -> full [N, 1]."""
        lvl = prep["lvl"]
        own_start = prep["own_start"]
        core_of, pos_of = prep["core_of"], prep["pos_of"]
        o = np.stack([np.asarray(outs[c]).reshape(-1) for c in range(NCORES)])
        out_full = np.empty((N, 1), np.float32)
        out_full[:, 0] = o[0, 0]
        upd = (lvl >= 1) & (lvl < L)
        vi = np.flatnonzero(upd)
        cols = np.array(own_start)[lvl[vi]] + pos_of[vi] + 1
        out_full[vi, 0] = o[core_of[vi], cols]
        return out_full

    return nc, in_maps, assemble


def build_exec(nc, in_maps):
    """Compile the program once into a reusable jitted executable with
    device-resident inputs (run_bass_kernel_spmd re-traces and re-uploads on
    every call, which dominates wall time under the axon tunnel)."""
    import jax
    from jax.sharding import Mesh, PartitionSpec, NamedSharding
    try:
        from jax.experimental.shard_map import shard_map
    except ImportError:
        from jax import shard_map
    from concourse import bass2jax
    from concourse.bass2jax import _bass_exec_p, partition_id_tensor

    bass2jax.install_neuronx_cc_hook()
    n_cores = NCORES
    pname = nc.partition_id_tensor.name if nc.partition_id_tensor else None
    in_names, out_names, out_avals, zero_outs = [], [], [], []
    for alloc in nc.m.functions[0].allocations:
        if not isinstance(alloc, mybir.MemoryLocationSet):
            continue
        name = alloc.memorylocations[0].name
        if alloc.kind == "ExternalInput":
            if name != pname:
                in_names.append(name)
        elif alloc.kind == "ExternalOutput":
            shape = tuple(alloc.tensor_shape)
            dtype = mybir.dt.np(alloc.dtype)
            out_avals.append(jax.core.ShapedArray(shape, dtype))
            zero_outs.append(np.zeros(shape, dtype))
            out_names.append(name)
    n_params = len(in_names)
    n_outs = len(out_avals)
    in_names_full = in_names + out_names + ([pname] if pname else [])

    def _body(*args):
        operands = list(args)
        if pname is not None:
            operands.append(partition_id_tensor())
        outs = _bass_exec_p.bind(
            *operands, out_avals=tuple(out_avals),
            in_names=tuple(in_names_full), out_names=tuple(out_names),
            lowering_input_output_aliases=(), sim_require_finite=True,
            sim_require_nnan=True, nc=nc)
        return tuple(outs)

    devices = jax.devices()[:n_cores]
    mesh = Mesh(np.asarray(devices), ("core",))
    in_specs = (PartitionSpec("core"),) * (n_params + n_outs)
    out_specs = (PartitionSpec("core"),) * len(out_names)
    donate = tuple(range(n_params, n_params + n_outs))
    sharded = jax.jit(shard_map(_body, mesh=mesh, in_specs=in_specs,
                                out_specs=out_specs, check_rep=False),
                      donate_argnums=donate, keep_unused=True)
    per_core = [[np.asarray(m[name]) for name in in_names] for m in in_maps]
    concat_in = [np.concatenate([per_core[c][i] for c in range(n_cores)],
                                axis=0) for i in range(n_params)]
    compiled = sharded.lower(
        *concat_in,
        *[np.zeros((n_cores * z.shape[0], *z.shape[1:]), z.dtype)
          for z in zero_outs]).compile()
    sh = NamedSharding(mesh, PartitionSpec("core"))
    dev_in = [jax.device_put(a, sh) for a in concat_in]
    jax.block_until_ready(dev_in)
    return dict(compiled=compiled, dev_in=dev_in, zero_outs=zero_outs,
                out_names=out_names, out_avals=out_avals, sh=sh,
                n_cores=n_cores)


def fresh_zeros(B):
    """Donated output buffers (device-resident) for one execution."""
    import jax
    zs = [jax.device_put(
        np.zeros((B["n_cores"] * z.shape[0], *z.shape[1:]), z.dtype), B["sh"])
        for z in B["zero_outs"]]
    jax.block_until_ready(zs)
    return zs


def run_exec(B, zs=None):
    """One kernel execution; returns per-core output dicts."""
    import jax
    if zs is None:
        zs = fresh_zeros(B)
    outs = B["compiled"](*B["dev_in"], *zs)
    jax.block_until_ready(outs)
    return [
        {name: np.asarray(outs[i]).reshape(B["n_cores"],
                                           *B["out_avals"][i].shape)[c]
         for i, name in enumerate(B["out_names"])}
        for c in range(B["n_cores"])
    ]


_CACHE = {}


def kernel(**inputs):
    key = tuple(sorted((k, tuple(np.asarray(v).shape))
                       for k, v in inputs.items()))
    ent = _CACHE.get(key)
    digest = None
    try:
        import hashlib
        h = hashlib.sha1()
        for k in sorted(inputs):
            h.update(np.ascontiguousarray(np.asarray(inputs[k])).tobytes())
        digest = h.hexdigest()
    except Exception:
        pass
    if ent is None or ent["digest"] != digest or digest is None:
        nc, in_maps, assemble = prepare(**inputs)
        try:
            B = build_exec(nc, in_maps)
        except Exception:
            B = None
        ent = {"nc": nc, "in_maps": in_maps, "assemble": assemble, "B": B,
               "digest": digest}
        _CACHE[key] = ent
    globals()["LAST_RUN"] = {
        "nc": ent["nc"], "in_maps": ent["in_maps"], "exec_time_ns": None,
        "B": ent["B"],
    }
    if ent["B"] is not None:
        outs = run_exec(ent["B"])
        return ent["assemble"]([outs[c]["outT"] for c in range(NCORES)])
    res = run_bass_kernel_spmd(ent["nc"], ent["in_maps"],
                               core_ids=list(range(NCORES)))
    return ent["assemble"]([res.results[c]["outT"] for c in range(NCORES)])
